# revision 13
# baseline (speedup 1.0000x reference)
"""Trainium2 Bass kernel for EnhancedConditionalUNet forward (B=64, 8 cores data-parallel).

Self-contained: hardcodes all shapes. kernel(**inputs) -> np.ndarray [64,3,64,64] f32.

Design: per-sample software pipeline on each core (8 samples/core).
- conv chain in fp16 matmuls with fp32 PSUM accumulation:
  e1 im2col; e2 stride-2 via even/odd x-planes with K-packed tap pairs on a
  DMA-duplicated (y-shifted) h1p copy; b1 full-K; b2 full-K;
  deconv as 2 M-packed phase-pairs (2 phases x 64ch = M=128, 6 windows);
  dc in x-parity layout (h5 stored as odd/even-x partition blocks) so tap
  pairs K-pack to 128 without duplication, outputs Q-packed via tile_position
  for a single 128-partition tanh per parity.
- attention entirely in fp8 e4m3 with DoubleRow matmuls (2 K-tiles via
  strided APs, 0.5 cyc/row): q/k/v convs, scores (j on partitions), colsum
  via fp8 ones, attnout; softmax denominators via one DVE approx reciprocal
  directly from PSUM; 1/colsum*gamma row-broadcast via PE replication.
- two-stage pipeline: sample s's front half (e1..attention) is emitted between
  sample s-1's normalization tail and back half (b2..dc).
"""
import numpy as np
import ml_dtypes

import concourse.bass as bass
import concourse.tile as tile
from concourse import bacc, mybir
from concourse.bass_utils import run_bass_kernel_spmd

NCORES = 8
NS = 8          # samples per core
BF = mybir.dt.bfloat16
F16 = mybir.dt.float16
F32 = mybir.dt.float32
F8 = mybir.dt.float8e4
AF = mybir.ActivationFunctionType
OP = mybir.AluOpType
DR = mybir.MatmulPerfMode.DoubleRow
NPF8 = ml_dtypes.float8_e4m3

_cache = {}


def build_nc(ns=NS):
    nc = bacc.Bacc("TRN2", target_bir_lowering=False, debug=False)

    d_m0 = nc.dram_tensor("m0", [ns, 36, 64, 64], F16, kind="ExternalInput")
    d_wim = nc.dram_tensor("wim", [36, 64], F16, kind="ExternalInput")
    d_be1 = nc.dram_tensor("be1", [64, 1], F32, kind="ExternalInput")
    d_we2k = nc.dram_tensor("we2k", [128, 6, 128], F16, kind="ExternalInput")
    d_be2 = nc.dram_tensor("be2", [128, 1], F32, kind="ExternalInput")
    d_wb1 = nc.dram_tensor("wb1", [128, 9, 2, 128], F16, kind="ExternalInput")
    d_bb1 = nc.dram_tensor("bb1", [128, 2], F32, kind="ExternalInput")
    d_wq8 = nc.dram_tensor("wq8", [128, 2, 128], F8, kind="ExternalInput")
    d_bq = nc.dram_tensor("bq", [32, 1], F32, kind="ExternalInput")
    d_wk8 = nc.dram_tensor("wk8", [128, 2, 128], F8, kind="ExternalInput")
    d_bk = nc.dram_tensor("bk", [32, 1], F32, kind="ExternalInput")
    d_wv8 = nc.dram_tensor("wv8", [128, 2, 256], F8, kind="ExternalInput")
    d_one8 = nc.dram_tensor("one8", [128, 2, 128], F8, kind="ExternalInput")
    d_gvb = nc.dram_tensor("gvb", [128, 2], F32, kind="ExternalInput")
    d_gam = nc.dram_tensor("gam", [1, 1], F32, kind="ExternalInput")
    d_wb2 = nc.dram_tensor("wb2", [128, 2, 9, 128], F16, kind="ExternalInput")
    d_bb2 = nc.dram_tensor("bb2", [128, 1], F32, kind="ExternalInput")
    d_wdtp = nc.dram_tensor("wdtp", [128, 2, 6, 128], F16, kind="ExternalInput")
    d_bdt2 = nc.dram_tensor("bdt2", [128, 1], F32, kind="ExternalInput")
    d_wdcp = nc.dram_tensor("wdcp", [128, 2, 6, 3], F16, kind="ExternalInput")
    d_bdc = nc.dram_tensor("bdc", [128, 1], F32, kind="ExternalInput")
    d_out = nc.dram_tensor("out", [ns, 3, 2, 64, 32], F32, kind="ExternalOutput")

    with tile.TileContext(nc) as tc:
        with (
            tc.tile_pool(name="wpool", bufs=1) as wp,
            tc.tile_pool(name="apool", bufs=2) as ap,
            tc.tile_pool(name="spool", bufs=1) as sp,
            tc.tile_pool(name="psS", bufs=4, space="PSUM") as psS,
            tc.tile_pool(name="psB", bufs=2, space="PSUM") as psB,
        ):
            _eng = [nc.gpsimd, nc.scalar]
            _ei = [0]

            def wload(name, shape, dt, dram, split=1):
                t = wp.tile(shape, dt, name=name)
                n0 = shape[0]
                step = (n0 + split - 1) // split
                for o in range(0, n0, step):
                    e = _eng[_ei[0] % len(_eng)]
                    _ei[0] += 1
                    e.dma_start(t[o:o + step], dram[o:o + step])
                return t

            wim = wload("wim", [36, 64], F16, d_wim)
            be1 = wload("be1", [64, 1], F32, d_be1)
            we2k = wload("we2k", [128, 6, 128], F16, d_we2k)
            be2 = wload("be2", [128, 1], F32, d_be2)
            wb1 = wload("wb1", [128, 9, 2, 128], F16, d_wb1, split=4)
            bb1 = wload("bb1", [128, 2], F32, d_bb1)
            wq8 = wload("wq8", [128, 2, 128], F8, d_wq8)
            bq = wload("bq", [32, 1], F32, d_bq)
            wk8 = wload("wk8", [128, 2, 128], F8, d_wk8)
            bk = wload("bk", [32, 1], F32, d_bk)
            wv8 = wload("wv8", [128, 2, 256], F8, d_wv8)
            one8 = wload("one8", [128, 2, 128], F8, d_one8)
            gvb = wload("gvb", [128, 2], F32, d_gvb)
            gam = wload("gam", [1, 1], F32, d_gam)
            ones1 = wp.tile([1, 128], BF)
            nc.vector.memset(ones1[:], 1.0)
            # persistent q/k fp8 tiles with a zeroed second DoubleRow K-slot
            qt = wp.tile([128, 2, 1024], F8, name="qt")
            kt = wp.tile([128, 2, 1024], F8, name="kt")
            nc.gpsimd.memset(qt[:], 0.0)
            nc.gpsimd.memset(kt[:], 0.0)

            taps = [(dy, dx) for dy in range(3) for dx in range(3)]

            def a_e1(s):
                """m0 load + e1 im2col conv -> h1pd both halves."""
                m0 = ap.tile([36, 64, 64], F16, name="m0t", bufs=3)
                _m0eng = [nc.sync, nc.gpsimd, nc.scalar, nc.sync]
                for t4 in range(4):
                    _m0eng[t4].dma_start(m0[9 * t4:9 * t4 + 9, :, :],
                                         d_m0[s, 9 * t4:9 * t4 + 9])
                h1pd = ap.tile([128, 66, 2, 33], F16, name="h1pd")
                nc.gpsimd.memset(h1pd[0:64, 0, :, :], 0.0)
                nc.gpsimd.memset(h1pd[0:64, 65, :, :], 0.0)
                nc.gpsimd.memset(h1pd[0:64, 1:65, 0, 0], 0.0)
                nc.gpsimd.memset(h1pd[0:64, 1:65, 1, 32], 0.0)
                nc.gpsimd.memset(h1pd[64:128, 64:66, :, :], 0.0)
                nc.gpsimd.memset(h1pd[64:128, 0:64, 0, 0], 0.0)
                nc.gpsimd.memset(h1pd[64:128, 0:64, 1, 32], 0.0)
                h1f = h1pd.rearrange("p a b c -> p a (b c)")
                for r in range(8):
                    ps = psS.tile([64, 512], F32, name="pcs")
                    nc.tensor.matmul(ps[:], wim[:], m0[:, 8 * r:8 * r + 8, :],
                                     start=True, stop=True)
                    # m0 cols pre-permuted on host: per row, first 32 -> plane0 xx1..32,
                    # last 32 -> plane1 xx0..31; flat row addr (p*33+xx) = 1..64 contiguous
                    # written twice: upper 64 partitions hold the same rows shifted
                    # one y up, giving e2's K-packed (dy,dy+1) tap pairs
                    pr = ps[:].rearrange("p (a b) -> p a b", a=8)
                    nc.scalar.activation(h1f[0:64, 1 + 8 * r:9 + 8 * r, 1:65],
                                         pr, AF.Relu, bias=be1[:], scale=1.0)
                    nc.vector.tensor_scalar(
                        out=h1f[64:128, 8 * r:8 + 8 * r, 1:65],
                        in0=pr, scalar1=be1[:], scalar2=0.0,
                        op0=OP.add, op1=OP.max)
                return dict(s=s, h1pd=h1pd)

            def a_e2(st):
                """e2: stride2 64->32, K-packed tap pairs (dy0+dy1) + singles (dy2).
                pass p<3: pair (0,p)+(1,p); pass p>=3: single (2,p-3), upper w=0"""
                h1pd = st["h1pd"]
                h2 = ap.tile([128, 34, 34], F16, name="h2")
                borders128(h2, 34, 34)
                for r in range(2):
                    ps = psS.tile([128, 512], F32, name="pcs")
                    for p in range(6):
                        dy, dx = (0, p) if p < 3 else (2, p - 3)
                        rhs = h1pd[:, dy + 32 * r:dy + 32 * r + 32:2,
                                   dx % 2, dx // 2:dx // 2 + 32]
                        nc.tensor.matmul(ps[:], we2k[:, p, :], rhs,
                                         start=(p == 0), stop=(p == 5))
                    nc.vector.tensor_scalar(out=h2[:, 1 + 16 * r:17 + 16 * r, 1:33],
                                            in0=ps[:], scalar1=be2[:], scalar2=0.0,
                                            op0=OP.add, op1=OP.max)
                st["h2"] = h2

            def a_b1(st):
                """b1: K=128, M=256 -> h3 [128,2,32,32] f16 + fp8 copy for q/k/v."""
                h2 = st["h2"]
                h3 = ap.tile([128, 2, 32, 32], F16, name="h3")
                for mh in range(2):
                    for r in range(2):
                        ps = psS.tile([128, 512], F32, name="pcs")
                        for ti, (dy, dx) in enumerate(taps):
                            nc.tensor.matmul(
                                ps[:], wb1[:, ti, mh, :],
                                h2[:, dy + 16 * r:dy + 16 * r + 16, dx:dx + 32],
                                start=(ti == 0), stop=(ti == 8))
                        nc.vector.tensor_scalar(
                            out=h3[:, mh, 16 * r:16 * r + 16, :].rearrange("p a b -> p (a b)"),
                            in0=ps[:], scalar1=bb1[:, mh:mh + 1], scalar2=0.0,
                            op0=OP.add, op1=OP.max)
                h3f = h3.rearrange("p m a b -> p m (a b)")
                h3q = ap.tile([128, 2, 1024], F8, name="h3q")
                nc.vector.tensor_copy(h3q[:, 0, :], h3f[:, 0, :])
                nc.scalar.activation(h3q[:, 1, :], h3f[:, 1, :], AF.Copy)
                st["h3f"] = h3f
                st["h3q"] = h3q

            def a_attn(st):
                """q/k/v convs (fp8 DoubleRow), scores S_T + exp -> E fp8."""
                h3q = st["h3q"]
                # q, k: fp8 DoubleRow over kh slots -> [32,1024] psum
                for (wt, bt, dst) in ((wq8, bq, qt), (wk8, bk, kt)):
                    psq = psB.tile([128, 1024], F32, name="pbig")
                    for c4 in range(4):
                        nc.tensor.matmul(psq[:, 256 * c4:256 * c4 + 256],
                                         wt[:], h3q[:, :, 256 * c4:256 * c4 + 256],
                                         start=True, stop=True, perf_mode=DR)
                    nc.vector.tensor_scalar(out=dst[0:32, 0, :], in0=psq[0:32, :],
                                            scalar1=bt[:], scalar2=None, op0=OP.add)

                # vT [128,8,256] fp8 via DoubleRow (lhsT = h3q j-slice)
                vT = ap.tile([128, 8, 256], F8, name="vT")
                for cc in range(8):
                    ps = psS.tile([128, 256], F32, name="pcs")
                    nc.tensor.matmul(ps[:], h3q[:, :, 128 * cc:128 * cc + 128],
                                     wv8[:], start=True, stop=True, perf_mode=DR)
                    nc.vector.tensor_copy(vT[:, cc, :], ps[:])

                # S_T + exp -> E fp8 (DoubleRow with zeroed second K-slot)
                E = ap.tile([128, 8, 1024], F8, name="E", bufs=1)
                for cc in range(8):
                    sps = psB.tile([128, 1024], F32, name="pbig")
                    for c4 in range(4):
                        nc.tensor.matmul(sps[:, 256 * c4:256 * c4 + 256],
                                         kt[:, :, 128 * cc:128 * cc + 128],
                                         qt[:, :, 256 * c4:256 * c4 + 256],
                                         start=True, stop=True, perf_mode=DR)
                    nc.scalar.activation(E[:, cc, :], sps[:], AF.Exp)
                st["E"] = E
                st["vT"] = vT

            def stage_a2(st):
                h3f, E, vT = st["h3f"], st["E"], st["vT"]
                # colsum via fp8 ones DoubleRow over cc pairs
                cs = psB.tile([128, 1024], F32, name="pbig")
                for c4 in range(4):
                    for cp in range(4):
                        nc.tensor.matmul(cs[:, 256 * c4:256 * c4 + 256], one8[:],
                                         E[:, 2 * cp:2 * cp + 2, 256 * c4:256 * c4 + 256],
                                         start=(cp == 0), stop=(cp == 3),
                                         perf_mode=DR)
                inv = sp.tile([1, 1024], F32, name="inv")
                nc.vector.reciprocal_approx_fast(out=inv[:], in_=cs[0:1, :])
                invg = ap.tile([1, 1024], BF, name="invg")
                nc.vector.tensor_scalar(out=invg[:], in0=inv[:], scalar1=gam[:],
                                        scalar2=None, op0=OP.mult)
                # attn out (v.E) per c-half -> atsb (f32, SBUF)
                atsb = ap.tile([128, 2, 1024], F32, name="atsb")
                for ch in range(2):
                    at = psB.tile([128, 1024], F32, name="pbig")
                    for c4 in range(4):
                        for cp in range(4):
                            nc.tensor.matmul(at[:, 256 * c4:256 * c4 + 256],
                                             vT[:, 2 * cp:2 * cp + 2, 128 * ch:128 * ch + 128],
                                             E[:, 2 * cp:2 * cp + 2, 256 * c4:256 * c4 + 256],
                                             start=(cp == 0), stop=(cp == 3),
                                             perf_mode=DR)
                    nc.vector.tensor_copy(atsb[:, ch, :], at[:])

                st["atsb"] = atsb
                st["invg"] = invg

            def emit_rep(st):
                invg = st["invg"]
                rep = psB.tile([128, 1024], F32, name="pbig")
                for ih in range(2):
                    nc.tensor.matmul(rep[:, 512 * ih:512 * ih + 512], ones1[:],
                                     invg[:, 512 * ih:512 * ih + 512],
                                     start=True, stop=True)
                repsb = sp.tile([128, 1024], F32, name="repsb")
                nc.vector.tensor_copy(repsb[:], rep[:])
                st["repsb"] = repsb

            def borders128(t, H, W):
                nc.gpsimd.memset(t[:, 0, :], 0.0)
                nc.gpsimd.memset(t[:, H - 1, :], 0.0)
                nc.gpsimd.memset(t[:, 1:H - 1, 0], 0.0)
                nc.gpsimd.memset(t[:, 1:H - 1, W - 1], 0.0)

            def stage_norm(st):
                """Apply 1/colsum*gamma + residual -> hb2 (rep precomputed or here)."""
                if "repsb" not in st:
                    emit_rep(st)
                atsb, h3f = st["atsb"], st["h3f"]
                repsb = st["repsb"]
                hb2 = ap.tile([128, 2, 34, 34], F16, name="hb2")
                for ch in range(2):
                    borders128(hb2[:, ch], 34, 34)
                    t1 = sp.tile([128, 1024], F32, name="t1")
                    nc.vector.tensor_mul(t1[:], atsb[:, ch, :], repsb[:])
                    nc.vector.scalar_tensor_tensor(
                        out=hb2[:, ch, 1:33, 1:33],
                        in0=t1[:], scalar=gvb[:, ch:ch + 1], in1=h3f[:, ch, :],
                        op0=OP.add, op1=OP.add)
                st["hb2"] = hb2

            def b_b2(st):
                """b2 conv on hb2 -> h4."""
                wb2, bb2 = late["wb2"], late["bb2"]
                hb2 = st["hb2"]
                h4 = ap.tile([128, 34, 34], F16, name="h4")
                borders128(h4, 34, 34)
                for r in range(2):
                    ps = psS.tile([128, 512], F32, name="pcs")
                    for kh in range(2):
                        for ti, (dy, dx) in enumerate(taps):
                            nc.tensor.matmul(
                                ps[:], wb2[:, kh, ti, :],
                                hb2[:, kh, dy + 16 * r:dy + 16 * r + 16, dx:dx + 32],
                                start=(kh == 0 and ti == 0), stop=(kh == 1 and ti == 8))
                    nc.vector.tensor_scalar(out=h4[:, 1 + 16 * r:17 + 16 * r, 1:33],
                                            in0=ps[:], scalar1=bb2[:], scalar2=0.0,
                                            op0=OP.add, op1=OP.max)
                st["h4"] = h4

            def b_dec(st):
                """deconv, M-packed phase pairs (px 0|1 stacked in M).
                h5x: [0:64]=odd-x (xx=(x-1)/2), [64:128]=even-x (xx=x/2)"""
                wdtp, bdt2 = late["wdtp"], late["bdt2"]
                h4 = st["h4"]
                h5x = ap.tile([128, 66, 33], F16, name="h5x")
                nc.gpsimd.memset(h5x[:, 0, :], 0.0)
                nc.gpsimd.memset(h5x[:, 65, :], 0.0)
                nc.gpsimd.memset(h5x[0:64, 1:65, 32], 0.0)
                nc.gpsimd.memset(h5x[64:128, 1:65, 0], 0.0)
                ays_all = ((1, 0), (2, 1))
                for py in range(2):
                    ays = ays_all[py]
                    for r in range(2):
                        ps = psS.tile([128, 512], F32, name="pcs")
                        for w6 in range(6):
                            iy, ax = divmod(w6, 3)
                            nc.tensor.matmul(
                                ps[:], wdtp[:, py, w6, :],
                                h4[:, ays[iy] + 16 * r:ays[iy] + 16 * r + 16, ax:ax + 32],
                                start=(w6 == 0), stop=(w6 == 5))
                        ys = slice(1 + py + 32 * r, 1 + py + 32 * r + 32, 2)
                        nc.scalar.activation(h5x[0:64, ys, 0:32], ps[0:64],
                                             AF.Relu, bias=bdt2[0:64], scale=1.0)
                        nc.vector.tensor_scalar(out=h5x[64:128, ys, 1:33],
                                                in0=ps[64:128], scalar1=bdt2[64:128],
                                                scalar2=0.0, op0=OP.add, op1=OP.max)
                st["h5x"] = h5x

            def b_dc(st):
                """dc in x-parity: 6 passes/parity (pair K=128 + single upper-only),
                Q-packed psum via tile_position, tanh, store."""
                wdcp, bdc = late["wdcp"], late["bdc"]
                s, h5x = st["s"], st["h5x"]
                dct = ap.tile([128, 2, 512], F32, name="dct")
                for par in range(2):
                    for Q in range(4):
                        pq = psS.tile([128, 512], F32, name="pcs")
                        for p6 in range(6):
                            dy, kind = divmod(p6, 2)
                            if par == 0:
                                xo = 0 if kind == 0 else 1
                            else:
                                xo = 1 if kind == 0 else 0
                            nc.tensor.matmul(
                                pq[32 * Q:32 * Q + 3, :],
                                wdcp[:, par, p6, :],
                                h5x[:, dy + 16 * Q:dy + 16 * Q + 16, xo:xo + 32],
                                start=(p6 == 0), stop=(p6 == 5),
                                tile_position=(0, 32 * Q))
                        nc.scalar.activation(dct[32 * Q:32 * Q + 3, par, :],
                                             pq[32 * Q:32 * Q + 3, :], AF.Tanh,
                                             bias=bdc[32 * Q:32 * Q + 3, :], scale=1.0)
                dctv = dct.rearrange("p q (a b) -> p q a b", a=16)
                for Q in range(4):
                    nc.sync.dma_start(d_out[s][:, :, 16 * Q:16 * Q + 16, :],
                                      dctv[32 * Q:32 * Q + 3])

            # emission order interleaves sample s's front half with sample
            # s-1's back half so the tensor queue always has independent work
            # while PSUM-evacuation chains (ACT/DVE) catch up
            prev = None
            late = {}
            for s in range(ns):
                if prev is not None:
                    stage_norm(prev)
                cur = a_e1(s)
                if s == 0:
                    # back-half weights: loaded while sample 0's front half runs
                    late["wb2"] = wload("wb2", [128, 2, 9, 128], F16, d_wb2, split=4)
                    late["bb2"] = wload("bb2", [128, 1], F32, d_bb2)
                    late["wdtp"] = wload("wdtp", [128, 2, 6, 128], F16, d_wdtp, split=2)
                    late["bdt2"] = wload("bdt2", [128, 1], F32, d_bdt2)
                    late["wdcp"] = wload("wdcp", [128, 2, 6, 3], F16, d_wdcp)
                    late["bdc"] = wload("bdc", [128, 1], F32, d_bdc)
                if prev is not None:
                    b_b2(prev)
                a_e2(cur)
                if prev is not None:
                    b_dec(prev)
                a_b1(cur)
                if prev is not None:
                    b_dc(prev)
                a_attn(cur)
                stage_a2(cur)
                if s == ns - 1:
                    emit_rep(cur)
                prev = cur
            stage_norm(prev)
            b_b2(prev)
            b_dec(prev)
            b_dc(prev)

    nc.compile()
    return nc


def _f8(a):
    return np.clip(a, -240, 240).astype(NPF8)


def prep_static(ew1, eb1, ew2, eb2, bw1, bb1, qw, qb, kw, kb, vw, vb,
                gamma, bw2, bb2, dtw, dtb, dcw, dcb):
    """Host-side weight layout prep (shared across cores)."""
    f16 = np.float16
    f32 = np.float32
    out = {}
    wim = np.zeros((36, 64), np.float32)
    for dy in range(3):
        for dx in range(3):
            t = dy * 3 + dx
            wim[t * 4:t * 4 + 4, :] = ew1[:, :, dy, dx].T
    out["wim"] = wim.astype(f16)
    out["be1"] = eb1.reshape(64, 1).astype(f32)
    we2 = np.ascontiguousarray(
        np.transpose(ew2, (1, 2, 3, 0)).reshape(64, 9, 128)).astype(np.float32)
    we2k = np.zeros((128, 6, 128), np.float32)
    for p in range(3):
        we2k[0:64, p, :] = we2[:, 0 * 3 + p, :]
        we2k[64:128, p, :] = we2[:, 1 * 3 + p, :]
        we2k[0:64, 3 + p, :] = we2[:, 2 * 3 + p, :]
    out["we2k"] = we2k.astype(f16)
    out["be2"] = eb2.reshape(128, 1).astype(f32)
    wb1 = np.transpose(bw1, (1, 2, 3, 0)).reshape(128, 9, 2, 128)
    out["wb1"] = np.ascontiguousarray(wb1).astype(f16)
    out["bb1"] = bb1.reshape(2, 128).T.astype(f32).copy()
    wq = qw[:, :, 0, 0].T.reshape(2, 128, 32).transpose(1, 0, 2)
    wqp = np.zeros((128, 2, 128), np.float32)
    wqp[:, :, 0:32] = wq
    out["wq8"] = _f8(wqp)
    out["bq"] = qb.reshape(32, 1).astype(f32)
    wk = kw[:, :, 0, 0].T.reshape(2, 128, 32).transpose(1, 0, 2)
    wkp = np.zeros((128, 2, 128), np.float32)
    wkp[:, :, 0:32] = wk
    out["wk8"] = _f8(wkp)
    out["bk"] = kb.reshape(32, 1).astype(f32)
    wv = vw[:, :, 0, 0].T.reshape(2, 128, 256).transpose(1, 0, 2)
    out["wv8"] = _f8(np.ascontiguousarray(wv))
    out["one8"] = np.ones((128, 2, 128), np.float32).astype(NPF8)
    g = float(np.asarray(gamma).reshape(-1)[0])
    out["gvb"] = (g * vb).reshape(2, 128).T.astype(f32).copy()
    out["gam"] = np.full((1, 1), g, f32)
    wb2_ = np.transpose(bw2, (1, 2, 3, 0)).reshape(2, 128, 9, 128).transpose(1, 0, 2, 3)
    out["wb2"] = np.ascontiguousarray(wb2_).astype(f16)
    out["bb2"] = bb2.reshape(128, 1).astype(f32)
    # deconv: M-packed phase pairs (py pairs px=0|1), 6 windows (ay,ax)
    kmap = {(0, 0): 1, (0, 1): 3, (1, 0): 0, (1, 1): 2}
    wdtp = np.zeros((128, 2, 6, 128), np.float32)
    for py in range(2):
        for iy in range(2):          # dy2 = iy; ay = aoff[(py, iy)]
            for ax in range(3):
                w6 = iy * 3 + ax
                # phase A (px=0): aoff[(0,dx2)]: dx2=0->ax1, dx2=1->ax0
                if ax == 1:
                    wdtp[:, py, w6, 0:64] = dtw[:, :, kmap[(py, iy)], kmap[(0, 0)]]
                elif ax == 0:
                    wdtp[:, py, w6, 0:64] = dtw[:, :, kmap[(py, iy)], kmap[(0, 1)]]
                # phase B (px=1): aoff[(1,dx2)]: dx2=0->ax2, dx2=1->ax1
                if ax == 2:
                    wdtp[:, py, w6, 64:128] = dtw[:, :, kmap[(py, iy)], kmap[(1, 0)]]
                elif ax == 1:
                    wdtp[:, py, w6, 64:128] = dtw[:, :, kmap[(py, iy)], kmap[(1, 1)]]
    out["wdtp"] = wdtp.astype(f16)
    out["bdt2"] = np.concatenate([dtb, dtb]).reshape(128, 1).astype(f32)
    # dc: x-parity passes; wdc[c, t, o] with t = dy*3+dx
    wdc = np.ascontiguousarray(
        np.transpose(dcw, (1, 2, 3, 0)).reshape(64, 9, 3)).astype(np.float32)
    wdcp = np.zeros((128, 2, 6, 3), np.float32)
    for dy in range(3):
        # par=0 (even x0): pair p=2dy: lower=tap(dy,1) odd, upper=tap(dy,0) even
        wdcp[0:64, 0, 2 * dy, :] = wdc[:, dy * 3 + 1, :]
        wdcp[64:128, 0, 2 * dy, :] = wdc[:, dy * 3 + 0, :]
        # par=0 single p=2dy+1: upper=tap(dy,2) even (xx offset 1)
        wdcp[64:128, 0, 2 * dy + 1, :] = wdc[:, dy * 3 + 2, :]
        # par=1 (odd x0): pair: lower=tap(dy,2) odd, upper=tap(dy,1) even (off 1)
        wdcp[0:64, 1, 2 * dy, :] = wdc[:, dy * 3 + 2, :]
        wdcp[64:128, 1, 2 * dy, :] = wdc[:, dy * 3 + 1, :]
        # par=1 single: lower=tap(dy,0) odd (off 0)
        wdcp[0:64, 1, 2 * dy + 1, :] = wdc[:, dy * 3 + 0, :]
    out["wdcp"] = wdcp.astype(f16)
    bdc = np.zeros((128, 1), f32)
    for Q in range(4):
        bdc[32 * Q:32 * Q + 3, 0] = dcb
    out["bdc"] = bdc
    return out


def pos_encoding():
    c = np.arange(2, dtype=np.float32)
    yy = np.arange(64, dtype=np.float32)
    ang = yy[None, :] / (10000.0 ** (2.0 * c / 4.0)).astype(np.float32)[:, None]
    pe = np.zeros((4, 64), np.float32)
    pe[0::2] = np.sin(ang)
    pe[1::2] = np.cos(ang)
    return pe


def build_m0(x_shard, le_shard):
    """x_shard [ns,3,64,64] f32, le_shard [ns,64,64] f32 -> [ns,36,64,64] f16."""
    ns = x_shard.shape[0]
    pe = pos_encoding()
    h0 = np.zeros((ns, 4, 66, 66), np.float32)
    h0[:, :3, 1:65, 1:65] = x_shard
    h0[:, 3, 1:65, 1:65] = le_shard
    h0[:, :, 1:65, 1:65] += pe[None, :, :, None]
    m0 = np.zeros((ns, 36, 64, 64), np.float32)
    for dy in range(3):
        for dx in range(3):
            t = dy * 3 + dx
            m0[:, t * 4:t * 4 + 4] = h0[:, :, dy:dy + 64, dx:dx + 64]
    # permute columns so e1's relu write is contiguous in the h1p plane layout:
    # first 32 cols -> odd x (plane0 slots xx1..32), last 32 -> even x (plane1 xx0..31)
    m0p = np.empty_like(m0)
    m0p[:, :, :, 0:32] = m0[:, :, :, 1::2]
    m0p[:, :, :, 32:64] = m0[:, :, :, 0::2]
    return m0p.astype(np.float16)


def make_in_maps(x, labels, label_emb, static):
    le = label_emb[labels].reshape(-1, 64, 64)
    in_maps = []
    for c in range(NCORES):
        sl = slice(c * NS, (c + 1) * NS)
        m = dict(static)
        m["m0"] = build_m0(x[sl], le[sl])
        in_maps.append(m)
    return in_maps


def kernel(x, t, labels, label_emb, ew1, eb1, ew2, eb2, bw1, bb1,
           qw, qb, kw, kb, vw, vb, gamma, bw2, bb2, dtw, dtb, dcw, dcb):
    del t
    x = np.asarray(x, np.float32)
    labels = np.asarray(labels)
    label_emb = np.asarray(label_emb, np.float32)
    static = prep_static(np.asarray(ew1), np.asarray(eb1), np.asarray(ew2),
                         np.asarray(eb2), np.asarray(bw1), np.asarray(bb1),
                         np.asarray(qw), np.asarray(qb), np.asarray(kw),
                         np.asarray(kb), np.asarray(vw), np.asarray(vb),
                         np.asarray(gamma), np.asarray(bw2), np.asarray(bb2),
                         np.asarray(dtw), np.asarray(dtb), np.asarray(dcw),
                         np.asarray(dcb))
    in_maps = make_in_maps(x, labels, label_emb, static)
    if "nc" not in _cache:
        _cache["nc"] = build_nc()
    nc = _cache["nc"]
    res = run_bass_kernel_spmd(nc, in_maps, core_ids=list(range(NCORES)))
    raw = np.concatenate([res.results[c]["out"] for c in range(NCORES)], axis=0)
    out = np.empty((raw.shape[0], 3, 64, 64), np.float32)
    out[:, :, :, 0::2] = raw[:, :, 0]
    out[:, :, :, 1::2] = raw[:, :, 1]
    return out


# revision 14
# speedup vs baseline: 1.0355x; 1.0355x over previous
"""Trainium2 Bass kernel for EnhancedConditionalUNet forward (B=64, 8 cores data-parallel).

Self-contained: hardcodes all shapes. kernel(**inputs) -> np.ndarray [64,3,64,64] f32.

Design: per-sample software pipeline on each core (8 samples/core).
- conv chain in fp16 matmuls with fp32 PSUM accumulation:
  e1 im2col; e2 stride-2 via even/odd x-planes with K-packed tap pairs on a
  DMA-duplicated (y-shifted) h1p copy; b1 full-K; b2 full-K;
  deconv as 2 M-packed phase-pairs (2 phases x 64ch = M=128, 6 windows);
  dc in x-parity layout (h5 stored as odd/even-x partition blocks) so tap
  pairs K-pack to 128 without duplication, outputs Q-packed via tile_position
  for a single 128-partition tanh per parity.
- attention entirely in fp8 e4m3 with DoubleRow matmuls (2 K-tiles via
  strided APs, 0.5 cyc/row): q/k/v convs, scores (j on partitions), colsum
  via fp8 ones, attnout; softmax denominators via one DVE approx reciprocal
  directly from PSUM; 1/colsum*gamma row-broadcast via PE replication.
- two-stage pipeline: sample s's front half (e1..attention) is emitted between
  sample s-1's normalization tail and back half (b2..dc).
"""
import numpy as np
import ml_dtypes

import concourse.bass as bass
import concourse.tile as tile
from concourse import bacc, mybir
from concourse.bass_utils import run_bass_kernel_spmd

NCORES = 8
NS = 8          # samples per core
BF = mybir.dt.bfloat16
F16 = mybir.dt.float16
F32 = mybir.dt.float32
F8 = mybir.dt.float8e4
AF = mybir.ActivationFunctionType
OP = mybir.AluOpType
DR = mybir.MatmulPerfMode.DoubleRow
NPF8 = ml_dtypes.float8_e4m3

_cache = {}


def build_nc(ns=NS):
    nc = bacc.Bacc("TRN2", target_bir_lowering=False, debug=False)

    d_m0 = nc.dram_tensor("m0", [ns, 36, 64, 64], F16, kind="ExternalInput")
    d_wim = nc.dram_tensor("wim", [36, 64], F16, kind="ExternalInput")
    d_be1 = nc.dram_tensor("be1", [64, 1], F32, kind="ExternalInput")
    d_we2k = nc.dram_tensor("we2k", [128, 6, 128], F16, kind="ExternalInput")
    d_be2 = nc.dram_tensor("be2", [128, 1], F32, kind="ExternalInput")
    d_wb1 = nc.dram_tensor("wb1", [128, 9, 2, 128], F16, kind="ExternalInput")
    d_bb1 = nc.dram_tensor("bb1", [128, 2], F32, kind="ExternalInput")
    d_wq8 = nc.dram_tensor("wq8", [128, 2, 128], F8, kind="ExternalInput")
    d_bq = nc.dram_tensor("bq", [32, 1], F32, kind="ExternalInput")
    d_wk8 = nc.dram_tensor("wk8", [128, 2, 128], F8, kind="ExternalInput")
    d_bk = nc.dram_tensor("bk", [32, 1], F32, kind="ExternalInput")
    d_wv8 = nc.dram_tensor("wv8", [128, 2, 256], F8, kind="ExternalInput")
    d_one8 = nc.dram_tensor("one8", [128, 2, 128], F8, kind="ExternalInput")
    d_gvb = nc.dram_tensor("gvb", [128, 2], F32, kind="ExternalInput")
    d_gam = nc.dram_tensor("gam", [1, 1], F32, kind="ExternalInput")
    d_wb2 = nc.dram_tensor("wb2", [128, 2, 9, 128], F16, kind="ExternalInput")
    d_bb2 = nc.dram_tensor("bb2", [128, 1], F32, kind="ExternalInput")
    d_wdtp = nc.dram_tensor("wdtp", [128, 2, 6, 128], F16, kind="ExternalInput")
    d_bdt2 = nc.dram_tensor("bdt2", [128, 1], F32, kind="ExternalInput")
    d_wdcp = nc.dram_tensor("wdcp", [128, 2, 6, 3], F16, kind="ExternalInput")
    d_bdc = nc.dram_tensor("bdc", [128, 1], F32, kind="ExternalInput")
    d_out = nc.dram_tensor("out", [ns, 3, 2, 64, 32], F32, kind="ExternalOutput")

    with tile.TileContext(nc) as tc:
        with (
            tc.tile_pool(name="wpool", bufs=1) as wp,
            tc.tile_pool(name="apool", bufs=2) as ap,
            tc.tile_pool(name="spool", bufs=1) as sp,
            tc.tile_pool(name="psS", bufs=4, space="PSUM") as psS,
            tc.tile_pool(name="psB", bufs=2, space="PSUM") as psB,
        ):
            _eng = [nc.gpsimd, nc.scalar]
            _ei = [0]

            def wload(name, shape, dt, dram, split=1):
                t = wp.tile(shape, dt, name=name)
                n0 = shape[0]
                step = (n0 + split - 1) // split
                for o in range(0, n0, step):
                    e = _eng[_ei[0] % len(_eng)]
                    _ei[0] += 1
                    e.dma_start(t[o:o + step], dram[o:o + step])
                return t

            wim = wload("wim", [36, 64], F16, d_wim)
            be1 = wload("be1", [64, 1], F32, d_be1)
            we2k = wload("we2k", [128, 6, 128], F16, d_we2k)
            be2 = wload("be2", [128, 1], F32, d_be2)
            wb1 = wload("wb1", [128, 9, 2, 128], F16, d_wb1, split=4)
            bb1 = wload("bb1", [128, 2], F32, d_bb1)
            wq8 = wload("wq8", [128, 2, 128], F8, d_wq8)
            bq = wload("bq", [32, 1], F32, d_bq)
            wk8 = wload("wk8", [128, 2, 128], F8, d_wk8)
            bk = wload("bk", [32, 1], F32, d_bk)
            wv8 = wload("wv8", [128, 2, 256], F8, d_wv8)
            one8 = wload("one8", [128, 2, 128], F8, d_one8)
            gvb = wload("gvb", [128, 2], F32, d_gvb)
            gam = wload("gam", [1, 1], F32, d_gam)
            ones1 = wp.tile([1, 128], BF)
            nc.vector.memset(ones1[:], 1.0)
            # persistent q/k fp8 tiles with a zeroed second DoubleRow K-slot
            qt = wp.tile([128, 2, 1024], F8, name="qt")
            kt = wp.tile([128, 2, 1024], F8, name="kt")
            nc.gpsimd.memset(qt[:], 0.0)
            nc.gpsimd.memset(kt[:], 0.0)

            taps = [(dy, dx) for dy in range(3) for dx in range(3)]

            def a_e1(s):
                """m0 load + e1 im2col conv -> h1pd both halves."""
                m0 = ap.tile([36, 64, 64], F16, name="m0t", bufs=3)
                _m0eng = [nc.sync, nc.gpsimd, nc.scalar, nc.sync]
                for t4 in range(4):
                    _m0eng[t4].dma_start(m0[9 * t4:9 * t4 + 9, :, :],
                                         d_m0[s, 9 * t4:9 * t4 + 9])
                h1pd = ap.tile([128, 66, 2, 33], F16, name="h1pd")
                nc.gpsimd.memset(h1pd[0:64, 0, :, :], 0.0)
                nc.gpsimd.memset(h1pd[0:64, 65, :, :], 0.0)
                nc.gpsimd.memset(h1pd[0:64, 1:65, 0, 0], 0.0)
                nc.gpsimd.memset(h1pd[0:64, 1:65, 1, 32], 0.0)
                nc.gpsimd.memset(h1pd[64:128, 64:66, :, :], 0.0)
                nc.gpsimd.memset(h1pd[64:128, 0:64, 0, 0], 0.0)
                nc.gpsimd.memset(h1pd[64:128, 0:64, 1, 32], 0.0)
                h1f = h1pd.rearrange("p a b c -> p a (b c)")
                for r in range(8):
                    ps = psS.tile([64, 512], F32, name="pcs")
                    nc.tensor.matmul(ps[:], wim[:], m0[:, 8 * r:8 * r + 8, :],
                                     start=True, stop=True)
                    # m0 cols pre-permuted on host: per row, first 32 -> plane0 xx1..32,
                    # last 32 -> plane1 xx0..31; flat row addr (p*33+xx) = 1..64 contiguous
                    # written twice: upper 64 partitions hold the same rows shifted
                    # one y up, giving e2's K-packed (dy,dy+1) tap pairs
                    pr = ps[:].rearrange("p (a b) -> p a b", a=8)
                    nc.scalar.activation(h1f[0:64, 1 + 8 * r:9 + 8 * r, 1:65],
                                         pr, AF.Relu, bias=be1[:], scale=1.0)
                    nc.scalar.activation(h1f[64:128, 8 * r:8 + 8 * r, 1:65],
                                         pr, AF.Relu, bias=be1[:], scale=1.0)
                return dict(s=s, h1pd=h1pd)

            def a_e2(st):
                """e2: stride2 64->32, K-packed tap pairs (dy0+dy1) + singles (dy2).
                pass p<3: pair (0,p)+(1,p); pass p>=3: single (2,p-3), upper w=0"""
                h1pd = st["h1pd"]
                h2 = ap.tile([128, 34, 34], F16, name="h2")
                borders128(h2, 34, 34)
                for r in range(2):
                    ps = psS.tile([128, 512], F32, name="pcs")
                    for i, p in enumerate((3, 4, 5, 0, 1, 2)):
                        dy, dx = (0, p) if p < 3 else (2, p - 3)
                        rhs = h1pd[:, dy + 32 * r:dy + 32 * r + 32:2,
                                   dx % 2, dx // 2:dx // 2 + 32]
                        if p >= 3:
                            # singles: K=64, lower half only (no dep on upper)
                            nc.tensor.matmul(ps[:], we2k[0:64, p, :], rhs[0:64],
                                             start=(i == 0), stop=(i == 5))
                        else:
                            nc.tensor.matmul(ps[:], we2k[:, p, :], rhs,
                                             start=(i == 0), stop=(i == 5))
                    nc.vector.tensor_scalar(out=h2[:, 1 + 16 * r:17 + 16 * r, 1:33],
                                            in0=ps[:], scalar1=be2[:], scalar2=0.0,
                                            op0=OP.add, op1=OP.max)
                st["h2"] = h2

            def a_b1(st):
                """b1: K=128, M=256 -> h3 [128,2,32,32] f16 + fp8 copy for q/k/v."""
                h2 = st["h2"]
                h3 = ap.tile([128, 2, 32, 32], F16, name="h3")
                for mh in range(2):
                    for r in range(2):
                        ps = psS.tile([128, 512], F32, name="pcs")
                        for ti, (dy, dx) in enumerate(taps):
                            nc.tensor.matmul(
                                ps[:], wb1[:, ti, mh, :],
                                h2[:, dy + 16 * r:dy + 16 * r + 16, dx:dx + 32],
                                start=(ti == 0), stop=(ti == 8))
                        nc.vector.tensor_scalar(
                            out=h3[:, mh, 16 * r:16 * r + 16, :].rearrange("p a b -> p (a b)"),
                            in0=ps[:], scalar1=bb1[:, mh:mh + 1], scalar2=0.0,
                            op0=OP.add, op1=OP.max)
                h3f = h3.rearrange("p m a b -> p m (a b)")
                h3q = ap.tile([128, 2, 1024], F8, name="h3q")
                nc.vector.tensor_copy(h3q[:, 0, :], h3f[:, 0, :])
                nc.scalar.activation(h3q[:, 1, :], h3f[:, 1, :], AF.Copy)
                st["h3f"] = h3f
                st["h3q"] = h3q

            def a_attn(st, dcg=None):
                """q/k/v convs (fp8 DoubleRow), scores S_T + exp -> E fp8.
                dcg: prev sample's dc groups, interleaved to keep the tensor
                queue fed while the ACT exp chain paces S_T."""
                h3q = st["h3q"]
                if dcg:
                    dcg[0]()
                    dcg[1]()
                # q, k: fp8 DoubleRow over kh slots -> [32,1024] psum
                for (wt, bt, dst) in ((wq8, bq, qt), (wk8, bk, kt)):
                    psq = psB.tile([128, 1024], F32, name="pbig")
                    for c4 in range(4):
                        nc.tensor.matmul(psq[:, 256 * c4:256 * c4 + 256],
                                         wt[:], h3q[:, :, 256 * c4:256 * c4 + 256],
                                         start=True, stop=True, perf_mode=DR)
                    nc.vector.tensor_scalar(out=dst[0:32, 0, :], in0=psq[0:32, :],
                                            scalar1=bt[:], scalar2=None, op0=OP.add)

                # vT [128,8,256] fp8 via DoubleRow (lhsT = h3q j-slice)
                vT = ap.tile([128, 8, 256], F8, name="vT")
                for cc in range(8):
                    ps = psS.tile([128, 256], F32, name="pcs")
                    nc.tensor.matmul(ps[:], h3q[:, :, 128 * cc:128 * cc + 128],
                                     wv8[:], start=True, stop=True, perf_mode=DR)
                    nc.vector.tensor_copy(vT[:, cc, :], ps[:])

                # S_T + exp -> E fp8 (DoubleRow with zeroed second K-slot)
                E = ap.tile([128, 8, 1024], F8, name="E", bufs=1)
                for cc in range(8):
                    sps = psB.tile([128, 1024], F32, name="pbig")
                    for c4 in range(4):
                        nc.tensor.matmul(sps[:, 256 * c4:256 * c4 + 256],
                                         kt[:, :, 128 * cc:128 * cc + 128],
                                         qt[:, :, 256 * c4:256 * c4 + 256],
                                         start=True, stop=True, perf_mode=DR)
                    nc.scalar.activation(E[:, cc, :], sps[:], AF.Exp)
                    if dcg and cc < 6:
                        dcg[cc + 2]()
                st["E"] = E
                st["vT"] = vT

            def stage_a2(st):
                h3f, E, vT = st["h3f"], st["E"], st["vT"]
                # colsum via fp8 ones DoubleRow over cc pairs
                cs = psB.tile([128, 1024], F32, name="pbig")
                for c4 in range(4):
                    for cp in range(4):
                        nc.tensor.matmul(cs[:, 256 * c4:256 * c4 + 256], one8[:],
                                         E[:, 2 * cp:2 * cp + 2, 256 * c4:256 * c4 + 256],
                                         start=(cp == 0), stop=(cp == 3),
                                         perf_mode=DR)
                inv = sp.tile([1, 1024], F32, name="inv")
                nc.vector.reciprocal_approx_fast(out=inv[:], in_=cs[0:1, :])
                invg = ap.tile([1, 1024], BF, name="invg")
                nc.vector.tensor_scalar(out=invg[:], in0=inv[:], scalar1=gam[:],
                                        scalar2=None, op0=OP.mult)
                # attn out (v.E) per c-half -> atsb (f32, SBUF)
                atsb = ap.tile([128, 2, 1024], F32, name="atsb")
                for ch in range(2):
                    at = psB.tile([128, 1024], F32, name="pbig")
                    for c4 in range(4):
                        for cp in range(4):
                            nc.tensor.matmul(at[:, 256 * c4:256 * c4 + 256],
                                             vT[:, 2 * cp:2 * cp + 2, 128 * ch:128 * ch + 128],
                                             E[:, 2 * cp:2 * cp + 2, 256 * c4:256 * c4 + 256],
                                             start=(cp == 0), stop=(cp == 3),
                                             perf_mode=DR)
                    nc.vector.tensor_copy(atsb[:, ch, :], at[:])

                st["atsb"] = atsb
                st["invg"] = invg

            def emit_rep(st):
                invg = st["invg"]
                rep = psB.tile([128, 1024], F32, name="pbig")
                for ih in range(2):
                    nc.tensor.matmul(rep[:, 512 * ih:512 * ih + 512], ones1[:],
                                     invg[:, 512 * ih:512 * ih + 512],
                                     start=True, stop=True)
                repsb = sp.tile([128, 1024], F32, name="repsb")
                nc.vector.tensor_copy(repsb[:], rep[:])
                st["repsb"] = repsb

            def borders128(t, H, W):
                nc.gpsimd.memset(t[:, 0, :], 0.0)
                nc.gpsimd.memset(t[:, H - 1, :], 0.0)
                nc.gpsimd.memset(t[:, 1:H - 1, 0], 0.0)
                nc.gpsimd.memset(t[:, 1:H - 1, W - 1], 0.0)

            def stage_norm(st):
                """Apply 1/colsum*gamma + residual -> hb2 (rep precomputed or here)."""
                if "repsb" not in st:
                    emit_rep(st)
                atsb, h3f = st["atsb"], st["h3f"]
                repsb = st["repsb"]
                hb2 = ap.tile([128, 2, 34, 34], F16, name="hb2")
                for ch in range(2):
                    borders128(hb2[:, ch], 34, 34)
                    t1 = sp.tile([128, 1024], F32, name="t1")
                    nc.vector.tensor_mul(t1[:], atsb[:, ch, :], repsb[:])
                    nc.vector.scalar_tensor_tensor(
                        out=hb2[:, ch, 1:33, 1:33],
                        in0=t1[:], scalar=gvb[:, ch:ch + 1], in1=h3f[:, ch, :],
                        op0=OP.add, op1=OP.add)
                st["hb2"] = hb2

            def b_b2(st):
                """b2 conv on hb2 -> h4."""
                wb2, bb2 = late["wb2"], late["bb2"]
                hb2 = st["hb2"]
                h4 = ap.tile([128, 34, 34], F16, name="h4")
                borders128(h4, 34, 34)
                for r in range(2):
                    ps = psS.tile([128, 512], F32, name="pcs")
                    for kh in range(2):
                        for ti, (dy, dx) in enumerate(taps):
                            nc.tensor.matmul(
                                ps[:], wb2[:, kh, ti, :],
                                hb2[:, kh, dy + 16 * r:dy + 16 * r + 16, dx:dx + 32],
                                start=(kh == 0 and ti == 0), stop=(kh == 1 and ti == 8))
                    nc.vector.tensor_scalar(out=h4[:, 1 + 16 * r:17 + 16 * r, 1:33],
                                            in0=ps[:], scalar1=bb2[:], scalar2=0.0,
                                            op0=OP.add, op1=OP.max)
                st["h4"] = h4

            def b_dec(st):
                """deconv, M-packed phase pairs (px 0|1 stacked in M).
                h5x: [0:64]=odd-x (xx=(x-1)/2), [64:128]=even-x (xx=x/2)"""
                wdtp, bdt2 = late["wdtp"], late["bdt2"]
                h4 = st["h4"]
                h5x = ap.tile([128, 66, 33], F16, name="h5x")
                nc.gpsimd.memset(h5x[:, 0, :], 0.0)
                nc.gpsimd.memset(h5x[:, 65, :], 0.0)
                nc.gpsimd.memset(h5x[0:64, 1:65, 32], 0.0)
                nc.gpsimd.memset(h5x[64:128, 1:65, 0], 0.0)
                ays_all = ((1, 0), (2, 1))
                for py in range(2):
                    ays = ays_all[py]
                    for r in range(2):
                        ps = psS.tile([128, 512], F32, name="pcs")
                        for w6 in range(6):
                            iy, ax = divmod(w6, 3)
                            nc.tensor.matmul(
                                ps[:], wdtp[:, py, w6, :],
                                h4[:, ays[iy] + 16 * r:ays[iy] + 16 * r + 16, ax:ax + 32],
                                start=(w6 == 0), stop=(w6 == 5))
                        ys = slice(1 + py + 32 * r, 1 + py + 32 * r + 32, 2)
                        nc.scalar.activation(h5x[0:64, ys, 0:32], ps[0:64],
                                             AF.Relu, bias=bdt2[0:64], scale=1.0)
                        nc.vector.tensor_scalar(out=h5x[64:128, ys, 1:33],
                                                in0=ps[64:128], scalar1=bdt2[64:128],
                                                scalar2=0.0, op0=OP.add, op1=OP.max)
                st["h5x"] = h5x

            def b_dc_groups(st):
                """dc in x-parity: 6 passes/parity (pair K=128 + single upper-only),
                Q-packed psum via tile_position, tanh, store. Returns 8 closures
                (one per (par,Q) group) for interleaving into a_attn."""
                wdcp, bdc = late["wdcp"], late["bdc"]
                s, h5x = st["s"], st["h5x"]
                dct = ap.tile([128, 2, 512], F32, name="dct")
                dctv = dct.rearrange("p q (a b) -> p q a b", a=16)

                def grp(par, Q):
                    pq = psS.tile([128, 512], F32, name="pcs")
                    for p6 in range(6):
                        dy, kind = divmod(p6, 2)
                        if par == 0:
                            xo = 0 if kind == 0 else 1
                        else:
                            xo = 1 if kind == 0 else 0
                        nc.tensor.matmul(
                            pq[32 * Q:32 * Q + 3, :],
                            wdcp[:, par, p6, :],
                            h5x[:, dy + 16 * Q:dy + 16 * Q + 16, xo:xo + 32],
                            start=(p6 == 0), stop=(p6 == 5),
                            tile_position=(0, 32 * Q))
                    nc.scalar.activation(dct[32 * Q:32 * Q + 3, par, :],
                                         pq[32 * Q:32 * Q + 3, :], AF.Tanh,
                                         bias=bdc[32 * Q:32 * Q + 3, :], scale=1.0)
                    if par == 1:
                        nc.sync.dma_start(d_out[s][:, :, 16 * Q:16 * Q + 16, :],
                                          dctv[32 * Q:32 * Q + 3])

                return [lambda par=par, Q=Q: grp(par, Q)
                        for par in range(2) for Q in range(4)]

            # emission order interleaves sample s's front half with sample
            # s-1's back half so the tensor queue always has independent work
            # while PSUM-evacuation chains (ACT/DVE) catch up
            prev = None
            late = {}
            for s in range(ns):
                if prev is not None:
                    stage_norm(prev)
                cur = a_e1(s)
                if s == 0:
                    # back-half weights: loaded while sample 0's front half runs
                    late["wb2"] = wload("wb2", [128, 2, 9, 128], F16, d_wb2, split=4)
                    late["bb2"] = wload("bb2", [128, 1], F32, d_bb2)
                    late["wdtp"] = wload("wdtp", [128, 2, 6, 128], F16, d_wdtp, split=2)
                    late["bdt2"] = wload("bdt2", [128, 1], F32, d_bdt2)
                    late["wdcp"] = wload("wdcp", [128, 2, 6, 3], F16, d_wdcp)
                    late["bdc"] = wload("bdc", [128, 1], F32, d_bdc)
                if prev is not None:
                    b_b2(prev)
                a_e2(cur)
                if prev is not None:
                    b_dec(prev)
                a_b1(cur)
                dcg = b_dc_groups(prev) if prev is not None else None
                a_attn(cur, dcg)
                stage_a2(cur)
                if s == ns - 1:
                    emit_rep(cur)
                prev = cur
            stage_norm(prev)
            b_b2(prev)
            b_dec(prev)
            for g in b_dc_groups(prev):
                g()

    nc.compile()
    return nc


def _f8(a):
    return np.clip(a, -240, 240).astype(NPF8)


def prep_static(ew1, eb1, ew2, eb2, bw1, bb1, qw, qb, kw, kb, vw, vb,
                gamma, bw2, bb2, dtw, dtb, dcw, dcb):
    """Host-side weight layout prep (shared across cores)."""
    f16 = np.float16
    f32 = np.float32
    out = {}
    wim = np.zeros((36, 64), np.float32)
    for dy in range(3):
        for dx in range(3):
            t = dy * 3 + dx
            wim[t * 4:t * 4 + 4, :] = ew1[:, :, dy, dx].T
    out["wim"] = wim.astype(f16)
    out["be1"] = eb1.reshape(64, 1).astype(f32)
    we2 = np.ascontiguousarray(
        np.transpose(ew2, (1, 2, 3, 0)).reshape(64, 9, 128)).astype(np.float32)
    we2k = np.zeros((128, 6, 128), np.float32)
    for p in range(3):
        we2k[0:64, p, :] = we2[:, 0 * 3 + p, :]
        we2k[64:128, p, :] = we2[:, 1 * 3 + p, :]
        we2k[0:64, 3 + p, :] = we2[:, 2 * 3 + p, :]
    out["we2k"] = we2k.astype(f16)
    out["be2"] = eb2.reshape(128, 1).astype(f32)
    wb1 = np.transpose(bw1, (1, 2, 3, 0)).reshape(128, 9, 2, 128)
    out["wb1"] = np.ascontiguousarray(wb1).astype(f16)
    out["bb1"] = bb1.reshape(2, 128).T.astype(f32).copy()
    wq = qw[:, :, 0, 0].T.reshape(2, 128, 32).transpose(1, 0, 2)
    wqp = np.zeros((128, 2, 128), np.float32)
    wqp[:, :, 0:32] = wq
    out["wq8"] = _f8(wqp)
    out["bq"] = qb.reshape(32, 1).astype(f32)
    wk = kw[:, :, 0, 0].T.reshape(2, 128, 32).transpose(1, 0, 2)
    wkp = np.zeros((128, 2, 128), np.float32)
    wkp[:, :, 0:32] = wk
    out["wk8"] = _f8(wkp)
    out["bk"] = kb.reshape(32, 1).astype(f32)
    wv = vw[:, :, 0, 0].T.reshape(2, 128, 256).transpose(1, 0, 2)
    out["wv8"] = _f8(np.ascontiguousarray(wv))
    out["one8"] = np.ones((128, 2, 128), np.float32).astype(NPF8)
    g = float(np.asarray(gamma).reshape(-1)[0])
    out["gvb"] = (g * vb).reshape(2, 128).T.astype(f32).copy()
    out["gam"] = np.full((1, 1), g, f32)
    wb2_ = np.transpose(bw2, (1, 2, 3, 0)).reshape(2, 128, 9, 128).transpose(1, 0, 2, 3)
    out["wb2"] = np.ascontiguousarray(wb2_).astype(f16)
    out["bb2"] = bb2.reshape(128, 1).astype(f32)
    # deconv: M-packed phase pairs (py pairs px=0|1), 6 windows (ay,ax)
    kmap = {(0, 0): 1, (0, 1): 3, (1, 0): 0, (1, 1): 2}
    wdtp = np.zeros((128, 2, 6, 128), np.float32)
    for py in range(2):
        for iy in range(2):          # dy2 = iy; ay = aoff[(py, iy)]
            for ax in range(3):
                w6 = iy * 3 + ax
                # phase A (px=0): aoff[(0,dx2)]: dx2=0->ax1, dx2=1->ax0
                if ax == 1:
                    wdtp[:, py, w6, 0:64] = dtw[:, :, kmap[(py, iy)], kmap[(0, 0)]]
                elif ax == 0:
                    wdtp[:, py, w6, 0:64] = dtw[:, :, kmap[(py, iy)], kmap[(0, 1)]]
                # phase B (px=1): aoff[(1,dx2)]: dx2=0->ax2, dx2=1->ax1
                if ax == 2:
                    wdtp[:, py, w6, 64:128] = dtw[:, :, kmap[(py, iy)], kmap[(1, 0)]]
                elif ax == 1:
                    wdtp[:, py, w6, 64:128] = dtw[:, :, kmap[(py, iy)], kmap[(1, 1)]]
    out["wdtp"] = wdtp.astype(f16)
    out["bdt2"] = np.concatenate([dtb, dtb]).reshape(128, 1).astype(f32)
    # dc: x-parity passes; wdc[c, t, o] with t = dy*3+dx
    wdc = np.ascontiguousarray(
        np.transpose(dcw, (1, 2, 3, 0)).reshape(64, 9, 3)).astype(np.float32)
    wdcp = np.zeros((128, 2, 6, 3), np.float32)
    for dy in range(3):
        # par=0 (even x0): pair p=2dy: lower=tap(dy,1) odd, upper=tap(dy,0) even
        wdcp[0:64, 0, 2 * dy, :] = wdc[:, dy * 3 + 1, :]
        wdcp[64:128, 0, 2 * dy, :] = wdc[:, dy * 3 + 0, :]
        # par=0 single p=2dy+1: upper=tap(dy,2) even (xx offset 1)
        wdcp[64:128, 0, 2 * dy + 1, :] = wdc[:, dy * 3 + 2, :]
        # par=1 (odd x0): pair: lower=tap(dy,2) odd, upper=tap(dy,1) even (off 1)
        wdcp[0:64, 1, 2 * dy, :] = wdc[:, dy * 3 + 2, :]
        wdcp[64:128, 1, 2 * dy, :] = wdc[:, dy * 3 + 1, :]
        # par=1 single: lower=tap(dy,0) odd (off 0)
        wdcp[0:64, 1, 2 * dy + 1, :] = wdc[:, dy * 3 + 0, :]
    out["wdcp"] = wdcp.astype(f16)
    bdc = np.zeros((128, 1), f32)
    for Q in range(4):
        bdc[32 * Q:32 * Q + 3, 0] = dcb
    out["bdc"] = bdc
    return out


def pos_encoding():
    c = np.arange(2, dtype=np.float32)
    yy = np.arange(64, dtype=np.float32)
    ang = yy[None, :] / (10000.0 ** (2.0 * c / 4.0)).astype(np.float32)[:, None]
    pe = np.zeros((4, 64), np.float32)
    pe[0::2] = np.sin(ang)
    pe[1::2] = np.cos(ang)
    return pe


def build_m0(x_shard, le_shard):
    """x_shard [ns,3,64,64] f32, le_shard [ns,64,64] f32 -> [ns,36,64,64] f16."""
    ns = x_shard.shape[0]
    pe = pos_encoding()
    h0 = np.zeros((ns, 4, 66, 66), np.float32)
    h0[:, :3, 1:65, 1:65] = x_shard
    h0[:, 3, 1:65, 1:65] = le_shard
    h0[:, :, 1:65, 1:65] += pe[None, :, :, None]
    m0 = np.zeros((ns, 36, 64, 64), np.float32)
    for dy in range(3):
        for dx in range(3):
            t = dy * 3 + dx
            m0[:, t * 4:t * 4 + 4] = h0[:, :, dy:dy + 64, dx:dx + 64]
    # permute columns so e1's relu write is contiguous in the h1p plane layout:
    # first 32 cols -> odd x (plane0 slots xx1..32), last 32 -> even x (plane1 xx0..31)
    m0p = np.empty_like(m0)
    m0p[:, :, :, 0:32] = m0[:, :, :, 1::2]
    m0p[:, :, :, 32:64] = m0[:, :, :, 0::2]
    return m0p.astype(np.float16)


def make_in_maps(x, labels, label_emb, static):
    le = label_emb[labels].reshape(-1, 64, 64)
    in_maps = []
    for c in range(NCORES):
        sl = slice(c * NS, (c + 1) * NS)
        m = dict(static)
        m["m0"] = build_m0(x[sl], le[sl])
        in_maps.append(m)
    return in_maps


def kernel(x, t, labels, label_emb, ew1, eb1, ew2, eb2, bw1, bb1,
           qw, qb, kw, kb, vw, vb, gamma, bw2, bb2, dtw, dtb, dcw, dcb):
    del t
    x = np.asarray(x, np.float32)
    labels = np.asarray(labels)
    label_emb = np.asarray(label_emb, np.float32)
    static = prep_static(np.asarray(ew1), np.asarray(eb1), np.asarray(ew2),
                         np.asarray(eb2), np.asarray(bw1), np.asarray(bb1),
                         np.asarray(qw), np.asarray(qb), np.asarray(kw),
                         np.asarray(kb), np.asarray(vw), np.asarray(vb),
                         np.asarray(gamma), np.asarray(bw2), np.asarray(bb2),
                         np.asarray(dtw), np.asarray(dtb), np.asarray(dcw),
                         np.asarray(dcb))
    in_maps = make_in_maps(x, labels, label_emb, static)
    if "nc" not in _cache:
        _cache["nc"] = build_nc()
    nc = _cache["nc"]
    res = run_bass_kernel_spmd(nc, in_maps, core_ids=list(range(NCORES)))
    raw = np.concatenate([res.results[c]["out"] for c in range(NCORES)], axis=0)
    out = np.empty((raw.shape[0], 3, 64, 64), np.float32)
    out[:, :, :, 0::2] = raw[:, :, 0]
    out[:, :, :, 1::2] = raw[:, :, 1]
    return out


# revision 15
# speedup vs baseline: 1.0387x; 1.0031x over previous
"""Trainium2 Bass kernel for EnhancedConditionalUNet forward (B=64, 8 cores data-parallel).

Self-contained: hardcodes all shapes. kernel(**inputs) -> np.ndarray [64,3,64,64] f32.

Design: per-sample software pipeline on each core (8 samples/core).
- conv chain in fp16 matmuls with fp32 PSUM accumulation:
  e1 im2col; e2 stride-2 via even/odd x-planes with K-packed tap pairs on a
  DMA-duplicated (y-shifted) h1p copy; b1 full-K; b2 full-K;
  deconv as 2 M-packed phase-pairs (2 phases x 64ch = M=128, 6 windows);
  dc in x-parity layout (h5 stored as odd/even-x partition blocks) so tap
  pairs K-pack to 128 without duplication, outputs Q-packed via tile_position
  for a single 128-partition tanh per parity.
- attention entirely in fp8 e4m3 with DoubleRow matmuls (2 K-tiles via
  strided APs, 0.5 cyc/row): q/k/v convs, scores (j on partitions), colsum
  via fp8 ones, attnout; softmax denominators via one DVE approx reciprocal
  directly from PSUM; 1/colsum*gamma row-broadcast via PE replication.
- two-stage pipeline: sample s's front half (e1..attention) is emitted between
  sample s-1's normalization tail and back half (b2..dc).
"""
import numpy as np
import ml_dtypes

import concourse.bass as bass
import concourse.tile as tile
from concourse import bacc, mybir
from concourse.bass_utils import run_bass_kernel_spmd

NCORES = 8
NS = 8          # samples per core
BF = mybir.dt.bfloat16
F16 = mybir.dt.float16
F32 = mybir.dt.float32
F8 = mybir.dt.float8e4
AF = mybir.ActivationFunctionType
OP = mybir.AluOpType
DR = mybir.MatmulPerfMode.DoubleRow
NPF8 = ml_dtypes.float8_e4m3

_cache = {}


def build_nc(ns=NS):
    nc = bacc.Bacc("TRN2", target_bir_lowering=False, debug=False)

    d_m0 = nc.dram_tensor("m0", [ns, 36, 64, 64], F16, kind="ExternalInput")
    d_wim = nc.dram_tensor("wim", [36, 64], F16, kind="ExternalInput")
    d_be1 = nc.dram_tensor("be1", [64, 1], F32, kind="ExternalInput")
    d_we2k = nc.dram_tensor("we2k", [128, 6, 128], F16, kind="ExternalInput")
    d_be2 = nc.dram_tensor("be2", [128, 1], F32, kind="ExternalInput")
    d_wb1 = nc.dram_tensor("wb1", [128, 9, 2, 128], F16, kind="ExternalInput")
    d_bb1 = nc.dram_tensor("bb1", [128, 2], F32, kind="ExternalInput")
    d_wq8 = nc.dram_tensor("wq8", [128, 2, 128], F8, kind="ExternalInput")
    d_bq = nc.dram_tensor("bq", [32, 1], F32, kind="ExternalInput")
    d_wk8 = nc.dram_tensor("wk8", [128, 2, 128], F8, kind="ExternalInput")
    d_bk = nc.dram_tensor("bk", [32, 1], F32, kind="ExternalInput")
    d_wv8 = nc.dram_tensor("wv8", [128, 2, 256], F8, kind="ExternalInput")
    d_one8 = nc.dram_tensor("one8", [128, 2, 128], F8, kind="ExternalInput")
    d_gvb = nc.dram_tensor("gvb", [128, 2], F32, kind="ExternalInput")
    d_gam = nc.dram_tensor("gam", [1, 1], F32, kind="ExternalInput")
    d_wb2 = nc.dram_tensor("wb2", [128, 2, 9, 128], F16, kind="ExternalInput")
    d_bb2 = nc.dram_tensor("bb2", [128, 1], F32, kind="ExternalInput")
    d_wdtp = nc.dram_tensor("wdtp", [128, 2, 6, 128], F16, kind="ExternalInput")
    d_bdt2 = nc.dram_tensor("bdt2", [128, 1], F32, kind="ExternalInput")
    d_wdcp = nc.dram_tensor("wdcp", [128, 2, 6, 3], F16, kind="ExternalInput")
    d_bdc = nc.dram_tensor("bdc", [128, 1], F32, kind="ExternalInput")
    d_out = nc.dram_tensor("out", [ns, 3, 2, 64, 32], F32, kind="ExternalOutput")

    with tile.TileContext(nc) as tc:
        with (
            tc.tile_pool(name="wpool", bufs=1) as wp,
            tc.tile_pool(name="apool", bufs=2) as ap,
            tc.tile_pool(name="spool", bufs=1) as sp,
            tc.tile_pool(name="psS", bufs=4, space="PSUM") as psS,
            tc.tile_pool(name="psB", bufs=2, space="PSUM") as psB,
        ):
            _eng = [nc.gpsimd, nc.scalar]
            _ei = [0]

            def wload(name, shape, dt, dram, split=1):
                t = wp.tile(shape, dt, name=name)
                n0 = shape[0]
                step = (n0 + split - 1) // split
                for o in range(0, n0, step):
                    e = _eng[_ei[0] % len(_eng)]
                    _ei[0] += 1
                    e.dma_start(t[o:o + step], dram[o:o + step])
                return t

            wim = wload("wim", [36, 64], F16, d_wim)
            be1 = wload("be1", [64, 1], F32, d_be1)
            we2k = wload("we2k", [128, 6, 128], F16, d_we2k)
            be2 = wload("be2", [128, 1], F32, d_be2)
            wb1 = wload("wb1", [128, 9, 2, 128], F16, d_wb1, split=4)
            bb1 = wload("bb1", [128, 2], F32, d_bb1)
            wq8 = wload("wq8", [128, 2, 128], F8, d_wq8)
            bq = wload("bq", [32, 1], F32, d_bq)
            wk8 = wload("wk8", [128, 2, 128], F8, d_wk8)
            bk = wload("bk", [32, 1], F32, d_bk)
            wv8 = wload("wv8", [128, 2, 256], F8, d_wv8)
            one8 = wload("one8", [128, 2, 128], F8, d_one8)
            gvb = wload("gvb", [128, 2], F32, d_gvb)
            gam = wload("gam", [1, 1], F32, d_gam)
            ones1 = wp.tile([1, 128], BF)
            nc.vector.memset(ones1[:], 1.0)
            # persistent q/k fp8 tiles with a zeroed second DoubleRow K-slot
            qt = wp.tile([128, 2, 1024], F8, name="qt")
            kt = wp.tile([128, 2, 1024], F8, name="kt")
            nc.gpsimd.memset(qt[:], 0.0)
            nc.gpsimd.memset(kt[:], 0.0)

            taps = [(dy, dx) for dy in range(3) for dx in range(3)]

            def a_e1(s):
                """m0 load + e1 im2col conv -> h1pd both halves."""
                m0 = ap.tile([36, 64, 64], F16, name="m0t", bufs=3)
                _m0eng = [nc.sync, nc.gpsimd, nc.scalar, nc.sync]
                for t4 in range(4):
                    _m0eng[t4].dma_start(m0[9 * t4:9 * t4 + 9, :, :],
                                         d_m0[s, 9 * t4:9 * t4 + 9])
                h1pd = ap.tile([128, 66, 2, 33], F16, name="h1pd")
                nc.gpsimd.memset(h1pd[0:64, 0, :, :], 0.0)
                nc.gpsimd.memset(h1pd[0:64, 65, :, :], 0.0)
                nc.gpsimd.memset(h1pd[0:64, 1:65, 0, 0], 0.0)
                nc.gpsimd.memset(h1pd[0:64, 1:65, 1, 32], 0.0)
                nc.gpsimd.memset(h1pd[64:128, 64:66, :, :], 0.0)
                nc.gpsimd.memset(h1pd[64:128, 0:64, 0, 0], 0.0)
                nc.gpsimd.memset(h1pd[64:128, 0:64, 1, 32], 0.0)
                h1f = h1pd.rearrange("p a b c -> p a (b c)")
                for r in range(8):
                    ps = psS.tile([64, 512], F32, name="pcs")
                    nc.tensor.matmul(ps[:], wim[:], m0[:, 8 * r:8 * r + 8, :],
                                     start=True, stop=True)
                    # m0 cols pre-permuted on host: per row, first 32 -> plane0 xx1..32,
                    # last 32 -> plane1 xx0..31; flat row addr (p*33+xx) = 1..64 contiguous
                    # written twice: upper 64 partitions hold the same rows shifted
                    # one y up, giving e2's K-packed (dy,dy+1) tap pairs
                    pr = ps[:].rearrange("p (a b) -> p a b", a=8)
                    nc.scalar.activation(h1f[0:64, 1 + 8 * r:9 + 8 * r, 1:65],
                                         pr, AF.Relu, bias=be1[:], scale=1.0)
                    nc.scalar.activation(h1f[64:128, 8 * r:8 + 8 * r, 1:65],
                                         pr, AF.Relu, bias=be1[:], scale=1.0)
                return dict(s=s, h1pd=h1pd)

            def a_e2(st):
                """e2: stride2 64->32, K-packed tap pairs (dy0+dy1) + singles (dy2).
                pass p<3: pair (0,p)+(1,p); pass p>=3: single (2,p-3), upper w=0"""
                h1pd = st["h1pd"]
                h2 = ap.tile([128, 34, 34], F16, name="h2")
                borders128(h2, 34, 34)
                for r in range(2):
                    ps = psS.tile([128, 512], F32, name="pcs")
                    for i, p in enumerate((3, 4, 5, 0, 1, 2)):
                        dy, dx = (0, p) if p < 3 else (2, p - 3)
                        rhs = h1pd[:, dy + 32 * r:dy + 32 * r + 32:2,
                                   dx % 2, dx // 2:dx // 2 + 32]
                        if p >= 3:
                            # singles: K=64, lower half only (no dep on upper)
                            nc.tensor.matmul(ps[:], we2k[0:64, p, :], rhs[0:64],
                                             start=(i == 0), stop=(i == 5))
                        else:
                            nc.tensor.matmul(ps[:], we2k[:, p, :], rhs,
                                             start=(i == 0), stop=(i == 5))
                    nc.vector.tensor_scalar(out=h2[:, 1 + 16 * r:17 + 16 * r, 1:33],
                                            in0=ps[:], scalar1=be2[:], scalar2=0.0,
                                            op0=OP.add, op1=OP.max)
                st["h2"] = h2

            def a_b1(st):
                """b1: K=128, M=256 -> h3 [128,2,32,32] f16 + fp8 copy for q/k/v."""
                h2 = st["h2"]
                h3 = ap.tile([128, 2, 32, 32], F16, name="h3")
                for mh in range(2):
                    for r in range(2):
                        ps = psS.tile([128, 512], F32, name="pcs")
                        for ti, (dy, dx) in enumerate(taps):
                            nc.tensor.matmul(
                                ps[:], wb1[:, ti, mh, :],
                                h2[:, dy + 16 * r:dy + 16 * r + 16, dx:dx + 32],
                                start=(ti == 0), stop=(ti == 8))
                        nc.vector.tensor_scalar(
                            out=h3[:, mh, 16 * r:16 * r + 16, :].rearrange("p a b -> p (a b)"),
                            in0=ps[:], scalar1=bb1[:, mh:mh + 1], scalar2=0.0,
                            op0=OP.add, op1=OP.max)
                h3f = h3.rearrange("p m a b -> p m (a b)")
                h3q = ap.tile([128, 2, 1024], F8, name="h3q")
                nc.vector.tensor_copy(h3q[:, 0, :], h3f[:, 0, :])
                nc.scalar.activation(h3q[:, 1, :], h3f[:, 1, :], AF.Copy)
                st["h3f"] = h3f
                st["h3q"] = h3q

            def a_attn(st, dcg=None):
                """q/k/v convs (fp8 DoubleRow), scores S_T + exp -> E fp8.
                dcg: prev sample's dc groups, interleaved to keep the tensor
                queue fed while the ACT exp chain paces S_T."""
                h3q = st["h3q"]
                if dcg:
                    dcg[0]()
                    dcg[1]()
                # q, k: fp8 DoubleRow over kh slots -> [32,1024] psum
                for (wt, bt, dst) in ((wq8, bq, qt), (wk8, bk, kt)):
                    psq = psB.tile([128, 1024], F32, name="pbig")
                    for c4 in range(4):
                        nc.tensor.matmul(psq[:, 256 * c4:256 * c4 + 256],
                                         wt[:], h3q[:, :, 256 * c4:256 * c4 + 256],
                                         start=True, stop=True, perf_mode=DR)
                    nc.vector.tensor_scalar(out=dst[0:32, 0, :], in0=psq[0:32, :],
                                            scalar1=bt[:], scalar2=None, op0=OP.add)

                # vT [128,8,256] fp8 via DoubleRow (lhsT = h3q j-slice)
                vT = ap.tile([128, 8, 256], F8, name="vT")
                for cc in range(8):
                    ps = psS.tile([128, 256], F32, name="pcs")
                    nc.tensor.matmul(ps[:], h3q[:, :, 128 * cc:128 * cc + 128],
                                     wv8[:], start=True, stop=True, perf_mode=DR)
                    nc.vector.tensor_copy(vT[:, cc, :], ps[:])

                # S_T + exp -> E fp8 (DoubleRow with zeroed second K-slot)
                E = ap.tile([128, 8, 1024], F8, name="E", bufs=1)
                for cc in range(8):
                    sps = psB.tile([128, 1024], F32, name="pbig")
                    for c4 in range(4):
                        nc.tensor.matmul(sps[:, 256 * c4:256 * c4 + 256],
                                         kt[:, :, 128 * cc:128 * cc + 128],
                                         qt[:, :, 256 * c4:256 * c4 + 256],
                                         start=True, stop=True, perf_mode=DR)
                    nc.scalar.activation(E[:, cc, :], sps[:], AF.Exp)
                    if dcg and cc < 6:
                        dcg[cc + 2]()
                st["E"] = E
                st["vT"] = vT

            def stage_a2(st):
                h3f, E, vT = st["h3f"], st["E"], st["vT"]
                # colsum via fp8 ones DoubleRow over cc pairs
                cs = psB.tile([128, 1024], F32, name="pbig")
                for c4 in range(4):
                    for cp in range(4):
                        nc.tensor.matmul(cs[:, 256 * c4:256 * c4 + 256], one8[:],
                                         E[:, 2 * cp:2 * cp + 2, 256 * c4:256 * c4 + 256],
                                         start=(cp == 0), stop=(cp == 3),
                                         perf_mode=DR)
                inv = sp.tile([1, 1024], F32, name="inv")
                nc.vector.reciprocal_approx_fast(out=inv[:], in_=cs[0:1, :])
                invg = ap.tile([1, 1024], BF, name="invg")
                nc.vector.tensor_scalar(out=invg[:], in0=inv[:], scalar1=gam[:],
                                        scalar2=None, op0=OP.mult)
                # attn out (v.E) per c-half -> atsb (f32, SBUF)
                atsb = ap.tile([128, 2, 1024], F32, name="atsb")
                for ch in range(2):
                    at = psB.tile([128, 1024], F32, name="pbig")
                    for c4 in range(4):
                        for cp in range(4):
                            nc.tensor.matmul(at[:, 256 * c4:256 * c4 + 256],
                                             vT[:, 2 * cp:2 * cp + 2, 128 * ch:128 * ch + 128],
                                             E[:, 2 * cp:2 * cp + 2, 256 * c4:256 * c4 + 256],
                                             start=(cp == 0), stop=(cp == 3),
                                             perf_mode=DR)
                    nc.vector.tensor_copy(atsb[:, ch, :], at[:])

                st["atsb"] = atsb
                st["invg"] = invg

            def emit_rep(st):
                invg = st["invg"]
                rep = psB.tile([128, 1024], F32, name="pbig")
                for ih in range(2):
                    nc.tensor.matmul(rep[:, 512 * ih:512 * ih + 512], ones1[:],
                                     invg[:, 512 * ih:512 * ih + 512],
                                     start=True, stop=True)
                repsb = sp.tile([128, 1024], F32, name="repsb")
                nc.vector.tensor_copy(repsb[:], rep[:])
                st["repsb"] = repsb

            def borders128(t, H, W):
                nc.gpsimd.memset(t[:, 0, :], 0.0)
                nc.gpsimd.memset(t[:, H - 1, :], 0.0)
                nc.gpsimd.memset(t[:, 1:H - 1, 0], 0.0)
                nc.gpsimd.memset(t[:, 1:H - 1, W - 1], 0.0)

            def stage_norm(st):
                """Apply 1/colsum*gamma + residual -> hb2 (rep precomputed or here)."""
                if "repsb" not in st:
                    emit_rep(st)
                atsb, h3f = st["atsb"], st["h3f"]
                repsb = st["repsb"]
                hb2 = ap.tile([128, 2, 34, 34], F16, name="hb2")
                for ch in range(2):
                    borders128(hb2[:, ch], 34, 34)
                    t1 = sp.tile([128, 1024], F32, name="t1")
                    nc.vector.tensor_mul(t1[:], atsb[:, ch, :], repsb[:])
                    nc.vector.scalar_tensor_tensor(
                        out=hb2[:, ch, 1:33, 1:33],
                        in0=t1[:], scalar=gvb[:, ch:ch + 1], in1=h3f[:, ch, :],
                        op0=OP.add, op1=OP.add)
                st["hb2"] = hb2

            def b_b2(st):
                """b2 conv on hb2 -> h4."""
                wb2, bb2 = late["wb2"], late["bb2"]
                hb2 = st["hb2"]
                h4 = ap.tile([128, 34, 34], F16, name="h4")
                borders128(h4, 34, 34)
                for r in range(2):
                    ps = psS.tile([128, 512], F32, name="pcs")
                    for kh in range(2):
                        for ti, (dy, dx) in enumerate(taps):
                            nc.tensor.matmul(
                                ps[:], wb2[:, kh, ti, :],
                                hb2[:, kh, dy + 16 * r:dy + 16 * r + 16, dx:dx + 32],
                                start=(kh == 0 and ti == 0), stop=(kh == 1 and ti == 8))
                    nc.vector.tensor_scalar(out=h4[:, 1 + 16 * r:17 + 16 * r, 1:33],
                                            in0=ps[:], scalar1=bb2[:], scalar2=0.0,
                                            op0=OP.add, op1=OP.max)
                st["h4"] = h4

            def b_dec(st):
                """deconv, M-packed phase pairs (px 0|1 stacked in M).
                h5x: [0:64]=odd-x (xx=(x-1)/2), [64:128]=even-x (xx=x/2)"""
                wdtp, bdt2 = late["wdtp"], late["bdt2"]
                h4 = st["h4"]
                h5x = ap.tile([128, 66, 33], F16, name="h5x")
                nc.gpsimd.memset(h5x[:, 0, :], 0.0)
                nc.gpsimd.memset(h5x[:, 65, :], 0.0)
                nc.gpsimd.memset(h5x[0:64, 1:65, 32], 0.0)
                nc.gpsimd.memset(h5x[64:128, 1:65, 0], 0.0)
                ays_all = ((1, 0), (2, 1))
                for py in range(2):
                    ays = ays_all[py]
                    for r in range(2):
                        ps = psS.tile([128, 512], F32, name="pcs")
                        for w6 in range(6):
                            iy, ax = divmod(w6, 3)
                            nc.tensor.matmul(
                                ps[:], wdtp[:, py, w6, :],
                                h4[:, ays[iy] + 16 * r:ays[iy] + 16 * r + 16, ax:ax + 32],
                                start=(w6 == 0), stop=(w6 == 5))
                        ys = slice(1 + py + 32 * r, 1 + py + 32 * r + 32, 2)
                        nc.scalar.activation(h5x[0:64, ys, 0:32], ps[0:64],
                                             AF.Relu, bias=bdt2[0:64], scale=1.0)
                        nc.vector.tensor_scalar(out=h5x[64:128, ys, 1:33],
                                                in0=ps[64:128], scalar1=bdt2[64:128],
                                                scalar2=0.0, op0=OP.add, op1=OP.max)
                st["h5x"] = h5x

            def b_dc_groups(st):
                """dc in x-parity: 6 passes/parity (pair K=128 + single upper-only),
                Q-packed psum via tile_position, tanh, store. Returns 8 closures
                (one per (par,Q) group) for interleaving into a_attn."""
                wdcp, bdc = late["wdcp"], late["bdc"]
                s, h5x = st["s"], st["h5x"]
                dct = ap.tile([128, 2, 512], F32, name="dct")
                dctv = dct.rearrange("p q (a b) -> p q a b", a=16)

                def grp(par, Q):
                    pq = psS.tile([128, 512], F32, name="pcs")
                    for p6 in range(6):
                        dy, kind = divmod(p6, 2)
                        if par == 0:
                            xo = 0 if kind == 0 else 1
                        else:
                            xo = 1 if kind == 0 else 0
                        nc.tensor.matmul(
                            pq[32 * Q:32 * Q + 3, :],
                            wdcp[:, par, p6, :],
                            h5x[:, dy + 16 * Q:dy + 16 * Q + 16, xo:xo + 32],
                            start=(p6 == 0), stop=(p6 == 5),
                            tile_position=(0, 32 * Q))
                    nc.scalar.activation(dct[32 * Q:32 * Q + 3, par, :],
                                         pq[32 * Q:32 * Q + 3, :], AF.Tanh,
                                         bias=bdc[32 * Q:32 * Q + 3, :], scale=1.0)
                    if par == 1:
                        nc.sync.dma_start(d_out[s][:, :, 16 * Q:16 * Q + 16, :],
                                          dctv[32 * Q:32 * Q + 3])

                return [lambda par=par, Q=Q: grp(par, Q)
                        for par in range(2) for Q in range(4)]

            # emission order interleaves sample s's front half with sample
            # s-1's back half so the tensor queue always has independent work
            # while PSUM-evacuation chains (ACT/DVE) catch up
            prev = None
            late = {}
            for s in range(ns):
                if prev is not None:
                    stage_norm(prev)
                cur = a_e1(s)
                if s == 0:
                    # back-half weights: loaded while sample 0's front half runs
                    late["wb2"] = wload("wb2", [128, 2, 9, 128], F16, d_wb2, split=4)
                    late["bb2"] = wload("bb2", [128, 1], F32, d_bb2)
                    late["wdtp"] = wload("wdtp", [128, 2, 6, 128], F16, d_wdtp, split=2)
                    late["bdt2"] = wload("bdt2", [128, 1], F32, d_bdt2)
                    late["wdcp"] = wload("wdcp", [128, 2, 6, 3], F16, d_wdcp)
                    late["bdc"] = wload("bdc", [128, 1], F32, d_bdc)
                a_e2(cur)
                a_b1(cur)
                a_attn(cur, None)
                if prev is not None:
                    b_b2(prev)
                    b_dec(prev)
                    for g in b_dc_groups(prev):
                        g()
                stage_a2(cur)
                if s == ns - 1:
                    emit_rep(cur)
                prev = cur
            stage_norm(prev)
            b_b2(prev)
            b_dec(prev)
            for g in b_dc_groups(prev):
                g()

    nc.compile()
    return nc


def _f8(a):
    return np.clip(a, -240, 240).astype(NPF8)


def prep_static(ew1, eb1, ew2, eb2, bw1, bb1, qw, qb, kw, kb, vw, vb,
                gamma, bw2, bb2, dtw, dtb, dcw, dcb):
    """Host-side weight layout prep (shared across cores)."""
    f16 = np.float16
    f32 = np.float32
    out = {}
    wim = np.zeros((36, 64), np.float32)
    for dy in range(3):
        for dx in range(3):
            t = dy * 3 + dx
            wim[t * 4:t * 4 + 4, :] = ew1[:, :, dy, dx].T
    out["wim"] = wim.astype(f16)
    out["be1"] = eb1.reshape(64, 1).astype(f32)
    we2 = np.ascontiguousarray(
        np.transpose(ew2, (1, 2, 3, 0)).reshape(64, 9, 128)).astype(np.float32)
    we2k = np.zeros((128, 6, 128), np.float32)
    for p in range(3):
        we2k[0:64, p, :] = we2[:, 0 * 3 + p, :]
        we2k[64:128, p, :] = we2[:, 1 * 3 + p, :]
        we2k[0:64, 3 + p, :] = we2[:, 2 * 3 + p, :]
    out["we2k"] = we2k.astype(f16)
    out["be2"] = eb2.reshape(128, 1).astype(f32)
    wb1 = np.transpose(bw1, (1, 2, 3, 0)).reshape(128, 9, 2, 128)
    out["wb1"] = np.ascontiguousarray(wb1).astype(f16)
    out["bb1"] = bb1.reshape(2, 128).T.astype(f32).copy()
    wq = qw[:, :, 0, 0].T.reshape(2, 128, 32).transpose(1, 0, 2)
    wqp = np.zeros((128, 2, 128), np.float32)
    wqp[:, :, 0:32] = wq
    out["wq8"] = _f8(wqp)
    out["bq"] = qb.reshape(32, 1).astype(f32)
    wk = kw[:, :, 0, 0].T.reshape(2, 128, 32).transpose(1, 0, 2)
    wkp = np.zeros((128, 2, 128), np.float32)
    wkp[:, :, 0:32] = wk
    out["wk8"] = _f8(wkp)
    out["bk"] = kb.reshape(32, 1).astype(f32)
    wv = vw[:, :, 0, 0].T.reshape(2, 128, 256).transpose(1, 0, 2)
    out["wv8"] = _f8(np.ascontiguousarray(wv))
    out["one8"] = np.ones((128, 2, 128), np.float32).astype(NPF8)
    g = float(np.asarray(gamma).reshape(-1)[0])
    out["gvb"] = (g * vb).reshape(2, 128).T.astype(f32).copy()
    out["gam"] = np.full((1, 1), g, f32)
    wb2_ = np.transpose(bw2, (1, 2, 3, 0)).reshape(2, 128, 9, 128).transpose(1, 0, 2, 3)
    out["wb2"] = np.ascontiguousarray(wb2_).astype(f16)
    out["bb2"] = bb2.reshape(128, 1).astype(f32)
    # deconv: M-packed phase pairs (py pairs px=0|1), 6 windows (ay,ax)
    kmap = {(0, 0): 1, (0, 1): 3, (1, 0): 0, (1, 1): 2}
    wdtp = np.zeros((128, 2, 6, 128), np.float32)
    for py in range(2):
        for iy in range(2):          # dy2 = iy; ay = aoff[(py, iy)]
            for ax in range(3):
                w6 = iy * 3 + ax
                # phase A (px=0): aoff[(0,dx2)]: dx2=0->ax1, dx2=1->ax0
                if ax == 1:
                    wdtp[:, py, w6, 0:64] = dtw[:, :, kmap[(py, iy)], kmap[(0, 0)]]
                elif ax == 0:
                    wdtp[:, py, w6, 0:64] = dtw[:, :, kmap[(py, iy)], kmap[(0, 1)]]
                # phase B (px=1): aoff[(1,dx2)]: dx2=0->ax2, dx2=1->ax1
                if ax == 2:
                    wdtp[:, py, w6, 64:128] = dtw[:, :, kmap[(py, iy)], kmap[(1, 0)]]
                elif ax == 1:
                    wdtp[:, py, w6, 64:128] = dtw[:, :, kmap[(py, iy)], kmap[(1, 1)]]
    out["wdtp"] = wdtp.astype(f16)
    out["bdt2"] = np.concatenate([dtb, dtb]).reshape(128, 1).astype(f32)
    # dc: x-parity passes; wdc[c, t, o] with t = dy*3+dx
    wdc = np.ascontiguousarray(
        np.transpose(dcw, (1, 2, 3, 0)).reshape(64, 9, 3)).astype(np.float32)
    wdcp = np.zeros((128, 2, 6, 3), np.float32)
    for dy in range(3):
        # par=0 (even x0): pair p=2dy: lower=tap(dy,1) odd, upper=tap(dy,0) even
        wdcp[0:64, 0, 2 * dy, :] = wdc[:, dy * 3 + 1, :]
        wdcp[64:128, 0, 2 * dy, :] = wdc[:, dy * 3 + 0, :]
        # par=0 single p=2dy+1: upper=tap(dy,2) even (xx offset 1)
        wdcp[64:128, 0, 2 * dy + 1, :] = wdc[:, dy * 3 + 2, :]
        # par=1 (odd x0): pair: lower=tap(dy,2) odd, upper=tap(dy,1) even (off 1)
        wdcp[0:64, 1, 2 * dy, :] = wdc[:, dy * 3 + 2, :]
        wdcp[64:128, 1, 2 * dy, :] = wdc[:, dy * 3 + 1, :]
        # par=1 single: lower=tap(dy,0) odd (off 0)
        wdcp[0:64, 1, 2 * dy + 1, :] = wdc[:, dy * 3 + 0, :]
    out["wdcp"] = wdcp.astype(f16)
    bdc = np.zeros((128, 1), f32)
    for Q in range(4):
        bdc[32 * Q:32 * Q + 3, 0] = dcb
    out["bdc"] = bdc
    return out


def pos_encoding():
    c = np.arange(2, dtype=np.float32)
    yy = np.arange(64, dtype=np.float32)
    ang = yy[None, :] / (10000.0 ** (2.0 * c / 4.0)).astype(np.float32)[:, None]
    pe = np.zeros((4, 64), np.float32)
    pe[0::2] = np.sin(ang)
    pe[1::2] = np.cos(ang)
    return pe


def build_m0(x_shard, le_shard):
    """x_shard [ns,3,64,64] f32, le_shard [ns,64,64] f32 -> [ns,36,64,64] f16."""
    ns = x_shard.shape[0]
    pe = pos_encoding()
    h0 = np.zeros((ns, 4, 66, 66), np.float32)
    h0[:, :3, 1:65, 1:65] = x_shard
    h0[:, 3, 1:65, 1:65] = le_shard
    h0[:, :, 1:65, 1:65] += pe[None, :, :, None]
    m0 = np.zeros((ns, 36, 64, 64), np.float32)
    for dy in range(3):
        for dx in range(3):
            t = dy * 3 + dx
            m0[:, t * 4:t * 4 + 4] = h0[:, :, dy:dy + 64, dx:dx + 64]
    # permute columns so e1's relu write is contiguous in the h1p plane layout:
    # first 32 cols -> odd x (plane0 slots xx1..32), last 32 -> even x (plane1 xx0..31)
    m0p = np.empty_like(m0)
    m0p[:, :, :, 0:32] = m0[:, :, :, 1::2]
    m0p[:, :, :, 32:64] = m0[:, :, :, 0::2]
    return m0p.astype(np.float16)


def make_in_maps(x, labels, label_emb, static):
    le = label_emb[labels].reshape(-1, 64, 64)
    in_maps = []
    for c in range(NCORES):
        sl = slice(c * NS, (c + 1) * NS)
        m = dict(static)
        m["m0"] = build_m0(x[sl], le[sl])
        in_maps.append(m)
    return in_maps


def kernel(x, t, labels, label_emb, ew1, eb1, ew2, eb2, bw1, bb1,
           qw, qb, kw, kb, vw, vb, gamma, bw2, bb2, dtw, dtb, dcw, dcb):
    del t
    x = np.asarray(x, np.float32)
    labels = np.asarray(labels)
    label_emb = np.asarray(label_emb, np.float32)
    static = prep_static(np.asarray(ew1), np.asarray(eb1), np.asarray(ew2),
                         np.asarray(eb2), np.asarray(bw1), np.asarray(bb1),
                         np.asarray(qw), np.asarray(qb), np.asarray(kw),
                         np.asarray(kb), np.asarray(vw), np.asarray(vb),
                         np.asarray(gamma), np.asarray(bw2), np.asarray(bb2),
                         np.asarray(dtw), np.asarray(dtb), np.asarray(dcw),
                         np.asarray(dcb))
    in_maps = make_in_maps(x, labels, label_emb, static)
    if "nc" not in _cache:
        _cache["nc"] = build_nc()
    nc = _cache["nc"]
    res = run_bass_kernel_spmd(nc, in_maps, core_ids=list(range(NCORES)))
    raw = np.concatenate([res.results[c]["out"] for c in range(NCORES)], axis=0)
    out = np.empty((raw.shape[0], 3, 64, 64), np.float32)
    out[:, :, :, 0::2] = raw[:, :, 0]
    out[:, :, :, 1::2] = raw[:, :, 1]
    return out


# revision 16
# speedup vs baseline: 1.0781x; 1.0379x over previous
"""Trainium2 Bass kernel for EnhancedConditionalUNet forward (B=64, 8 cores data-parallel).

Self-contained: hardcodes all shapes. kernel(**inputs) -> np.ndarray [64,3,64,64] f32.

Design: per-sample software pipeline on each core (8 samples/core).
- conv chain in fp16 matmuls with fp32 PSUM accumulation:
  e1 im2col; e2 stride-2 via even/odd x-planes with K-packed tap pairs on a
  DMA-duplicated (y-shifted) h1p copy; b1 full-K; b2 full-K;
  deconv as 2 M-packed phase-pairs (2 phases x 64ch = M=128, 6 windows);
  dc in x-parity layout (h5 stored as odd/even-x partition blocks) so tap
  pairs K-pack to 128 without duplication, outputs Q-packed via tile_position
  for a single 128-partition tanh per parity.
- attention entirely in fp8 e4m3 with DoubleRow matmuls (2 K-tiles via
  strided APs, 0.5 cyc/row): q/k/v convs, scores (j on partitions), colsum
  via fp8 ones, attnout; softmax denominators via one DVE approx reciprocal
  directly from PSUM; 1/colsum*gamma row-broadcast via PE replication.
- two-stage pipeline: sample s's front half (e1..attention) is emitted between
  sample s-1's normalization tail and back half (b2..dc).
"""
import numpy as np
import ml_dtypes

import concourse.bass as bass
import concourse.tile as tile
from concourse import bacc, mybir
from concourse.bass_utils import run_bass_kernel_spmd

NCORES = 8
NS = 8          # samples per core
BF = mybir.dt.bfloat16
F16 = mybir.dt.float16
F32 = mybir.dt.float32
F8 = mybir.dt.float8e4
AF = mybir.ActivationFunctionType
OP = mybir.AluOpType
DR = mybir.MatmulPerfMode.DoubleRow
NPF8 = ml_dtypes.float8_e4m3

_cache = {}


def build_nc(ns=NS):
    nc = bacc.Bacc("TRN2", target_bir_lowering=False, debug=False)

    d_m0 = nc.dram_tensor("m0", [ns, 36, 64, 64], F16, kind="ExternalInput")
    d_wim = nc.dram_tensor("wim", [36, 64], F16, kind="ExternalInput")
    d_be1 = nc.dram_tensor("be1", [64, 1], F32, kind="ExternalInput")
    d_we2k = nc.dram_tensor("we2k", [128, 6, 128], F16, kind="ExternalInput")
    d_be2 = nc.dram_tensor("be2", [128, 1], F32, kind="ExternalInput")
    d_wb1 = nc.dram_tensor("wb1", [128, 9, 2, 128], F16, kind="ExternalInput")
    d_bb1 = nc.dram_tensor("bb1", [128, 2], F32, kind="ExternalInput")
    d_wq8 = nc.dram_tensor("wq8", [128, 2, 128], F8, kind="ExternalInput")
    d_bq = nc.dram_tensor("bq", [32, 1], F32, kind="ExternalInput")
    d_wk8 = nc.dram_tensor("wk8", [128, 2, 128], F8, kind="ExternalInput")
    d_bk = nc.dram_tensor("bk", [32, 1], F32, kind="ExternalInput")
    d_wv8 = nc.dram_tensor("wv8", [128, 2, 256], F8, kind="ExternalInput")
    d_one8 = nc.dram_tensor("one8", [128, 2, 128], F8, kind="ExternalInput")
    d_gvb = nc.dram_tensor("gvb", [128, 2], F32, kind="ExternalInput")
    d_gam = nc.dram_tensor("gam", [1, 1], F32, kind="ExternalInput")
    d_wb2 = nc.dram_tensor("wb2", [128, 2, 9, 128], F16, kind="ExternalInput")
    d_bb2 = nc.dram_tensor("bb2", [128, 1], F32, kind="ExternalInput")
    d_wdtp = nc.dram_tensor("wdtp", [128, 2, 6, 128], F16, kind="ExternalInput")
    d_bdt2 = nc.dram_tensor("bdt2", [128, 1], F32, kind="ExternalInput")
    d_wdcp = nc.dram_tensor("wdcp", [128, 2, 6, 3], F16, kind="ExternalInput")
    d_bdc = nc.dram_tensor("bdc", [128, 1], F32, kind="ExternalInput")
    d_out = nc.dram_tensor("out", [ns, 3, 2, 64, 32], F32, kind="ExternalOutput")

    with tile.TileContext(nc) as tc:
        with (
            tc.tile_pool(name="wpool", bufs=1) as wp,
            tc.tile_pool(name="apool", bufs=2) as ap,
            tc.tile_pool(name="spool", bufs=1) as sp,
            tc.tile_pool(name="psS", bufs=4, space="PSUM") as psS,
            tc.tile_pool(name="psB", bufs=2, space="PSUM") as psB,
        ):
            _eng = [nc.gpsimd, nc.scalar]
            _ei = [0]

            def wload(name, shape, dt, dram, split=1):
                t = wp.tile(shape, dt, name=name)
                n0 = shape[0]
                step = (n0 + split - 1) // split
                for o in range(0, n0, step):
                    e = _eng[_ei[0] % len(_eng)]
                    _ei[0] += 1
                    e.dma_start(t[o:o + step], dram[o:o + step])
                return t

            wim = wload("wim", [36, 64], F16, d_wim)
            be1 = wload("be1", [64, 1], F32, d_be1)
            we2k = wload("we2k", [128, 6, 128], F16, d_we2k)
            be2 = wload("be2", [128, 1], F32, d_be2)
            wb1 = wload("wb1", [128, 9, 2, 128], F16, d_wb1, split=4)
            bb1 = wload("bb1", [128, 2], F32, d_bb1)
            wq8 = wload("wq8", [128, 2, 128], F8, d_wq8)
            bq = wload("bq", [32, 1], F32, d_bq)
            wk8 = wload("wk8", [128, 2, 128], F8, d_wk8)
            bk = wload("bk", [32, 1], F32, d_bk)
            wv8 = wload("wv8", [128, 2, 256], F8, d_wv8)
            one8 = wload("one8", [128, 2, 128], F8, d_one8)
            gvb = wload("gvb", [128, 2], F32, d_gvb)
            gam = wload("gam", [1, 1], F32, d_gam)
            ones1 = wp.tile([1, 128], BF)
            nc.vector.memset(ones1[:], 1.0)
            # persistent q/k fp8 tiles with a zeroed second DoubleRow K-slot
            qt = wp.tile([128, 2, 1024], F8, name="qt")
            kt = wp.tile([128, 2, 1024], F8, name="kt")
            nc.gpsimd.memset(qt[:], 0.0)
            nc.gpsimd.memset(kt[:], 0.0)

            taps = [(dy, dx) for dy in range(3) for dx in range(3)]

            def a_e1(s):
                """m0 load + e1 im2col conv -> h1pd both halves."""
                m0 = ap.tile([36, 64, 64], F16, name="m0t", bufs=3)
                _m0eng = [nc.sync, nc.gpsimd, nc.scalar, nc.sync]
                for t4 in range(4):
                    _m0eng[t4].dma_start(m0[9 * t4:9 * t4 + 9, :, :],
                                         d_m0[s, 9 * t4:9 * t4 + 9])
                h1pd = ap.tile([128, 66, 2, 33], F16, name="h1pd")
                nc.gpsimd.memset(h1pd[0:64, 0, :, :], 0.0)
                nc.gpsimd.memset(h1pd[0:64, 65, :, :], 0.0)
                nc.gpsimd.memset(h1pd[0:64, 1:65, 0, 0], 0.0)
                nc.gpsimd.memset(h1pd[0:64, 1:65, 1, 32], 0.0)
                nc.gpsimd.memset(h1pd[64:128, 64:66, :, :], 0.0)
                nc.gpsimd.memset(h1pd[64:128, 0:64, 0, 0], 0.0)
                nc.gpsimd.memset(h1pd[64:128, 0:64, 1, 32], 0.0)
                h1f = h1pd.rearrange("p a b c -> p a (b c)")
                for r in range(8):
                    ps = psS.tile([64, 512], F32, name="pcs")
                    nc.tensor.matmul(ps[:], wim[:], m0[:, 8 * r:8 * r + 8, :],
                                     start=True, stop=True)
                    # m0 cols pre-permuted on host: per row, first 32 -> plane0 xx1..32,
                    # last 32 -> plane1 xx0..31; flat row addr (p*33+xx) = 1..64 contiguous
                    # written twice: upper 64 partitions hold the same rows shifted
                    # one y up, giving e2's K-packed (dy,dy+1) tap pairs
                    pr = ps[:].rearrange("p (a b) -> p a b", a=8)
                    nc.scalar.activation(h1f[0:64, 1 + 8 * r:9 + 8 * r, 1:65],
                                         pr, AF.Relu, bias=be1[:], scale=1.0)
                    nc.scalar.activation(h1f[64:128, 8 * r:8 + 8 * r, 1:65],
                                         pr, AF.Relu, bias=be1[:], scale=1.0)
                return dict(s=s, h1pd=h1pd)

            def a_e2(st):
                """e2: stride2 64->32, K-packed tap pairs (dy0+dy1) + singles (dy2).
                pass p<3: pair (0,p)+(1,p); pass p>=3: single (2,p-3), upper w=0"""
                h1pd = st["h1pd"]
                h2 = ap.tile([128, 34, 34], F16, name="h2")
                borders128(h2, 34, 34)
                for r in range(2):
                    ps = psS.tile([128, 512], F32, name="pcs")
                    for p in range(6):
                        dy, dx = (0, p) if p < 3 else (2, p - 3)
                        rhs = h1pd[:, dy + 32 * r:dy + 32 * r + 32:2,
                                   dx % 2, dx // 2:dx // 2 + 32]
                        nc.tensor.matmul(ps[:], we2k[:, p, :], rhs,
                                         start=(p == 0), stop=(p == 5))
                    nc.vector.tensor_scalar(out=h2[:, 1 + 16 * r:17 + 16 * r, 1:33],
                                            in0=ps[:], scalar1=be2[:], scalar2=0.0,
                                            op0=OP.add, op1=OP.max)
                st["h2"] = h2

            def a_b1(st):
                """b1: K=128, M=256 -> h3 [128,2,32,32] f16 + fp8 copy for q/k/v."""
                h2 = st["h2"]
                h3 = ap.tile([128, 2, 32, 32], F16, name="h3")
                for mh in range(2):
                    for r in range(2):
                        ps = psS.tile([128, 512], F32, name="pcs")
                        for ti, (dy, dx) in enumerate(taps):
                            nc.tensor.matmul(
                                ps[:], wb1[:, ti, mh, :],
                                h2[:, dy + 16 * r:dy + 16 * r + 16, dx:dx + 32],
                                start=(ti == 0), stop=(ti == 8))
                        nc.vector.tensor_scalar(
                            out=h3[:, mh, 16 * r:16 * r + 16, :].rearrange("p a b -> p (a b)"),
                            in0=ps[:], scalar1=bb1[:, mh:mh + 1], scalar2=0.0,
                            op0=OP.add, op1=OP.max)
                h3f = h3.rearrange("p m a b -> p m (a b)")
                h3q = ap.tile([128, 2, 1024], F8, name="h3q")
                nc.vector.tensor_copy(h3q[:, 0, :], h3f[:, 0, :])
                nc.scalar.activation(h3q[:, 1, :], h3f[:, 1, :], AF.Copy)
                st["h3f"] = h3f
                st["h3q"] = h3q

            def a_attn(st, dcg=None):
                """q/k/v convs (fp8 DoubleRow), scores S_T + exp -> E fp8.
                dcg: prev sample's dc groups, interleaved to keep the tensor
                queue fed while the ACT exp chain paces S_T."""
                h3q = st["h3q"]
                if dcg:
                    dcg[0]()
                    dcg[1]()
                # q, k: fp8 DoubleRow over kh slots -> [32,1024] psum
                for (wt, bt, dst) in ((wq8, bq, qt), (wk8, bk, kt)):
                    psq = psB.tile([128, 1024], F32, name="pbig")
                    for c4 in range(4):
                        nc.tensor.matmul(psq[:, 256 * c4:256 * c4 + 256],
                                         wt[:], h3q[:, :, 256 * c4:256 * c4 + 256],
                                         start=True, stop=True, perf_mode=DR)
                    nc.vector.tensor_scalar(out=dst[0:32, 0, :], in0=psq[0:32, :],
                                            scalar1=bt[:], scalar2=None, op0=OP.add)

                # vT [128,8,256] fp8 via DoubleRow (lhsT = h3q j-slice)
                vT = ap.tile([128, 8, 256], F8, name="vT")
                for cc in range(8):
                    ps = psS.tile([128, 256], F32, name="pcs")
                    nc.tensor.matmul(ps[:], h3q[:, :, 128 * cc:128 * cc + 128],
                                     wv8[:], start=True, stop=True, perf_mode=DR)
                    nc.vector.tensor_copy(vT[:, cc, :], ps[:])

                # S_T + exp -> E fp8 (DoubleRow with zeroed second K-slot)
                E = ap.tile([128, 8, 1024], F8, name="E", bufs=1)
                for cc in range(8):
                    sps = psB.tile([128, 1024], F32, name="pbig")
                    for c4 in range(4):
                        nc.tensor.matmul(sps[:, 256 * c4:256 * c4 + 256],
                                         kt[:, :, 128 * cc:128 * cc + 128],
                                         qt[:, :, 256 * c4:256 * c4 + 256],
                                         start=True, stop=True, perf_mode=DR)
                    nc.scalar.activation(E[:, cc, :], sps[:], AF.Exp)
                    if dcg and cc < 6:
                        dcg[cc + 2]()
                st["E"] = E
                st["vT"] = vT

            def stage_a2(st):
                h3f, E, vT = st["h3f"], st["E"], st["vT"]
                # colsum via fp8 ones DoubleRow over cc pairs
                cs = psB.tile([128, 1024], F32, name="pbig")
                for c4 in range(4):
                    for cp in range(4):
                        nc.tensor.matmul(cs[:, 256 * c4:256 * c4 + 256], one8[:],
                                         E[:, 2 * cp:2 * cp + 2, 256 * c4:256 * c4 + 256],
                                         start=(cp == 0), stop=(cp == 3),
                                         perf_mode=DR)
                inv = sp.tile([1, 1024], F32, name="inv")
                nc.vector.reciprocal_approx_fast(out=inv[:], in_=cs[0:1, :])
                invg = ap.tile([1, 1024], BF, name="invg")
                nc.vector.tensor_scalar(out=invg[:], in0=inv[:], scalar1=gam[:],
                                        scalar2=None, op0=OP.mult)
                # attn out (v.E) per c-half -> atsb (f32, SBUF)
                atsb = ap.tile([128, 2, 1024], F32, name="atsb")
                for ch in range(2):
                    at = psB.tile([128, 1024], F32, name="pbig")
                    for c4 in range(4):
                        for cp in range(4):
                            nc.tensor.matmul(at[:, 256 * c4:256 * c4 + 256],
                                             vT[:, 2 * cp:2 * cp + 2, 128 * ch:128 * ch + 128],
                                             E[:, 2 * cp:2 * cp + 2, 256 * c4:256 * c4 + 256],
                                             start=(cp == 0), stop=(cp == 3),
                                             perf_mode=DR)
                    nc.vector.tensor_copy(atsb[:, ch, :], at[:])

                st["atsb"] = atsb
                st["invg"] = invg

            def emit_rep(st):
                invg = st["invg"]
                rep = psB.tile([128, 1024], F32, name="pbig")
                for ih in range(2):
                    nc.tensor.matmul(rep[:, 512 * ih:512 * ih + 512], ones1[:],
                                     invg[:, 512 * ih:512 * ih + 512],
                                     start=True, stop=True)
                repsb = sp.tile([128, 1024], F32, name="repsb")
                nc.vector.tensor_copy(repsb[:], rep[:])
                st["repsb"] = repsb

            def borders128(t, H, W):
                nc.gpsimd.memset(t[:, 0, :], 0.0)
                nc.gpsimd.memset(t[:, H - 1, :], 0.0)
                nc.gpsimd.memset(t[:, 1:H - 1, 0], 0.0)
                nc.gpsimd.memset(t[:, 1:H - 1, W - 1], 0.0)

            def stage_norm(st):
                """Apply 1/colsum*gamma + residual -> hb2 (rep precomputed or here)."""
                if "repsb" not in st:
                    emit_rep(st)
                atsb, h3f = st["atsb"], st["h3f"]
                repsb = st["repsb"]
                hb2 = ap.tile([128, 2, 34, 34], F16, name="hb2")
                for ch in range(2):
                    borders128(hb2[:, ch], 34, 34)
                    t1 = sp.tile([128, 1024], F32, name="t1")
                    nc.vector.tensor_mul(t1[:], atsb[:, ch, :], repsb[:])
                    nc.vector.scalar_tensor_tensor(
                        out=hb2[:, ch, 1:33, 1:33],
                        in0=t1[:], scalar=gvb[:, ch:ch + 1], in1=h3f[:, ch, :],
                        op0=OP.add, op1=OP.add)
                st["hb2"] = hb2

            def b_b2(st):
                """b2 conv on hb2 -> h4."""
                wb2, bb2 = late["wb2"], late["bb2"]
                hb2 = st["hb2"]
                h4 = ap.tile([128, 34, 34], F16, name="h4")
                borders128(h4, 34, 34)
                for r in range(2):
                    ps = psS.tile([128, 512], F32, name="pcs")
                    for kh in range(2):
                        for ti, (dy, dx) in enumerate(taps):
                            nc.tensor.matmul(
                                ps[:], wb2[:, kh, ti, :],
                                hb2[:, kh, dy + 16 * r:dy + 16 * r + 16, dx:dx + 32],
                                start=(kh == 0 and ti == 0), stop=(kh == 1 and ti == 8))
                    nc.vector.tensor_scalar(out=h4[:, 1 + 16 * r:17 + 16 * r, 1:33],
                                            in0=ps[:], scalar1=bb2[:], scalar2=0.0,
                                            op0=OP.add, op1=OP.max)
                st["h4"] = h4

            def b_dec(st):
                """deconv, M-packed phase pairs (px 0|1 stacked in M).
                h5x: [0:64]=odd-x (xx=(x-1)/2), [64:128]=even-x (xx=x/2)"""
                wdtp, bdt2 = late["wdtp"], late["bdt2"]
                h4 = st["h4"]
                h5x = ap.tile([128, 66, 33], F16, name="h5x")
                nc.gpsimd.memset(h5x[:, 0, :], 0.0)
                nc.gpsimd.memset(h5x[:, 65, :], 0.0)
                nc.gpsimd.memset(h5x[0:64, 1:65, 32], 0.0)
                nc.gpsimd.memset(h5x[64:128, 1:65, 0], 0.0)
                ays_all = ((1, 0), (2, 1))
                for py in range(2):
                    ays = ays_all[py]
                    for r in range(2):
                        ps = psS.tile([128, 512], F32, name="pcs")
                        for w6 in range(6):
                            iy, ax = divmod(w6, 3)
                            nc.tensor.matmul(
                                ps[:], wdtp[:, py, w6, :],
                                h4[:, ays[iy] + 16 * r:ays[iy] + 16 * r + 16, ax:ax + 32],
                                start=(w6 == 0), stop=(w6 == 5))
                        ys = slice(1 + py + 32 * r, 1 + py + 32 * r + 32, 2)
                        nc.scalar.activation(h5x[0:64, ys, 0:32], ps[0:64],
                                             AF.Relu, bias=bdt2[0:64], scale=1.0)
                        nc.vector.tensor_scalar(out=h5x[64:128, ys, 1:33],
                                                in0=ps[64:128], scalar1=bdt2[64:128],
                                                scalar2=0.0, op0=OP.add, op1=OP.max)
                st["h5x"] = h5x

            def b_dc_groups(st):
                """dc in x-parity: 6 passes/parity (pair K=128 + single upper-only),
                Q-packed psum via tile_position, tanh, store. Returns 8 closures
                (one per (par,Q) group) for interleaving into a_attn."""
                wdcp, bdc = late["wdcp"], late["bdc"]
                s, h5x = st["s"], st["h5x"]
                dct = ap.tile([128, 2, 512], F32, name="dct")
                dctv = dct.rearrange("p q (a b) -> p q a b", a=16)

                def grp(par):
                    pq = psS.tile([128, 512], F32, name="pcs")
                    for p6 in range(6):
                        dy, kind = divmod(p6, 2)
                        if par == 0:
                            xo = 0 if kind == 0 else 1
                        else:
                            xo = 1 if kind == 0 else 0
                        for Q in range(4):
                            nc.tensor.matmul(
                                pq[32 * Q:32 * Q + 3, :],
                                wdcp[:, par, p6, :],
                                h5x[:, dy + 16 * Q:dy + 16 * Q + 16, xo:xo + 32],
                                start=(p6 == 0), stop=(p6 == 5),
                                tile_position=(0, 32 * Q))
                    for Q in range(4):
                        nc.scalar.activation(dct[32 * Q:32 * Q + 3, par, :],
                                             pq[32 * Q:32 * Q + 3, :], AF.Tanh,
                                             bias=bdc[32 * Q:32 * Q + 3, :], scale=1.0)
                        if par == 1:
                            nc.sync.dma_start(d_out[s][:, :, 16 * Q:16 * Q + 16, :],
                                              dctv[32 * Q:32 * Q + 3])

                return [lambda par=par: grp(par) for par in range(2)]

            # emission order interleaves sample s's front half with sample
            # s-1's back half so the tensor queue always has independent work
            # while PSUM-evacuation chains (ACT/DVE) catch up
            prev = None
            late = {}
            for s in range(ns):
                if prev is not None:
                    stage_norm(prev)
                cur = a_e1(s)
                if s == 0:
                    # back-half weights: loaded while sample 0's front half runs
                    late["wb2"] = wload("wb2", [128, 2, 9, 128], F16, d_wb2, split=4)
                    late["bb2"] = wload("bb2", [128, 1], F32, d_bb2)
                    late["wdtp"] = wload("wdtp", [128, 2, 6, 128], F16, d_wdtp, split=2)
                    late["bdt2"] = wload("bdt2", [128, 1], F32, d_bdt2)
                    late["wdcp"] = wload("wdcp", [128, 2, 6, 3], F16, d_wdcp)
                    late["bdc"] = wload("bdc", [128, 1], F32, d_bdc)
                a_e2(cur)
                a_b1(cur)
                a_attn(cur, None)
                if prev is not None:
                    b_b2(prev)
                    b_dec(prev)
                    for g in b_dc_groups(prev):
                        g()
                stage_a2(cur)
                if s == ns - 1:
                    emit_rep(cur)
                prev = cur
            stage_norm(prev)
            b_b2(prev)
            b_dec(prev)
            for g in b_dc_groups(prev):
                g()

    nc.compile()
    return nc


def _f8(a):
    return np.clip(a, -240, 240).astype(NPF8)


def prep_static(ew1, eb1, ew2, eb2, bw1, bb1, qw, qb, kw, kb, vw, vb,
                gamma, bw2, bb2, dtw, dtb, dcw, dcb):
    """Host-side weight layout prep (shared across cores)."""
    f16 = np.float16
    f32 = np.float32
    out = {}
    wim = np.zeros((36, 64), np.float32)
    for dy in range(3):
        for dx in range(3):
            t = dy * 3 + dx
            wim[t * 4:t * 4 + 4, :] = ew1[:, :, dy, dx].T
    out["wim"] = wim.astype(f16)
    out["be1"] = eb1.reshape(64, 1).astype(f32)
    we2 = np.ascontiguousarray(
        np.transpose(ew2, (1, 2, 3, 0)).reshape(64, 9, 128)).astype(np.float32)
    we2k = np.zeros((128, 6, 128), np.float32)
    for p in range(3):
        we2k[0:64, p, :] = we2[:, 0 * 3 + p, :]
        we2k[64:128, p, :] = we2[:, 1 * 3 + p, :]
        we2k[0:64, 3 + p, :] = we2[:, 2 * 3 + p, :]
    out["we2k"] = we2k.astype(f16)
    out["be2"] = eb2.reshape(128, 1).astype(f32)
    wb1 = np.transpose(bw1, (1, 2, 3, 0)).reshape(128, 9, 2, 128)
    out["wb1"] = np.ascontiguousarray(wb1).astype(f16)
    out["bb1"] = bb1.reshape(2, 128).T.astype(f32).copy()
    wq = qw[:, :, 0, 0].T.reshape(2, 128, 32).transpose(1, 0, 2)
    wqp = np.zeros((128, 2, 128), np.float32)
    wqp[:, :, 0:32] = wq
    out["wq8"] = _f8(wqp)
    out["bq"] = qb.reshape(32, 1).astype(f32)
    wk = kw[:, :, 0, 0].T.reshape(2, 128, 32).transpose(1, 0, 2)
    wkp = np.zeros((128, 2, 128), np.float32)
    wkp[:, :, 0:32] = wk
    out["wk8"] = _f8(wkp)
    out["bk"] = kb.reshape(32, 1).astype(f32)
    wv = vw[:, :, 0, 0].T.reshape(2, 128, 256).transpose(1, 0, 2)
    out["wv8"] = _f8(np.ascontiguousarray(wv))
    out["one8"] = np.ones((128, 2, 128), np.float32).astype(NPF8)
    g = float(np.asarray(gamma).reshape(-1)[0])
    out["gvb"] = (g * vb).reshape(2, 128).T.astype(f32).copy()
    out["gam"] = np.full((1, 1), g, f32)
    wb2_ = np.transpose(bw2, (1, 2, 3, 0)).reshape(2, 128, 9, 128).transpose(1, 0, 2, 3)
    out["wb2"] = np.ascontiguousarray(wb2_).astype(f16)
    out["bb2"] = bb2.reshape(128, 1).astype(f32)
    # deconv: M-packed phase pairs (py pairs px=0|1), 6 windows (ay,ax)
    kmap = {(0, 0): 1, (0, 1): 3, (1, 0): 0, (1, 1): 2}
    wdtp = np.zeros((128, 2, 6, 128), np.float32)
    for py in range(2):
        for iy in range(2):          # dy2 = iy; ay = aoff[(py, iy)]
            for ax in range(3):
                w6 = iy * 3 + ax
                # phase A (px=0): aoff[(0,dx2)]: dx2=0->ax1, dx2=1->ax0
                if ax == 1:
                    wdtp[:, py, w6, 0:64] = dtw[:, :, kmap[(py, iy)], kmap[(0, 0)]]
                elif ax == 0:
                    wdtp[:, py, w6, 0:64] = dtw[:, :, kmap[(py, iy)], kmap[(0, 1)]]
                # phase B (px=1): aoff[(1,dx2)]: dx2=0->ax2, dx2=1->ax1
                if ax == 2:
                    wdtp[:, py, w6, 64:128] = dtw[:, :, kmap[(py, iy)], kmap[(1, 0)]]
                elif ax == 1:
                    wdtp[:, py, w6, 64:128] = dtw[:, :, kmap[(py, iy)], kmap[(1, 1)]]
    out["wdtp"] = wdtp.astype(f16)
    out["bdt2"] = np.concatenate([dtb, dtb]).reshape(128, 1).astype(f32)
    # dc: x-parity passes; wdc[c, t, o] with t = dy*3+dx
    wdc = np.ascontiguousarray(
        np.transpose(dcw, (1, 2, 3, 0)).reshape(64, 9, 3)).astype(np.float32)
    wdcp = np.zeros((128, 2, 6, 3), np.float32)
    for dy in range(3):
        # par=0 (even x0): pair p=2dy: lower=tap(dy,1) odd, upper=tap(dy,0) even
        wdcp[0:64, 0, 2 * dy, :] = wdc[:, dy * 3 + 1, :]
        wdcp[64:128, 0, 2 * dy, :] = wdc[:, dy * 3 + 0, :]
        # par=0 single p=2dy+1: upper=tap(dy,2) even (xx offset 1)
        wdcp[64:128, 0, 2 * dy + 1, :] = wdc[:, dy * 3 + 2, :]
        # par=1 (odd x0): pair: lower=tap(dy,2) odd, upper=tap(dy,1) even (off 1)
        wdcp[0:64, 1, 2 * dy, :] = wdc[:, dy * 3 + 2, :]
        wdcp[64:128, 1, 2 * dy, :] = wdc[:, dy * 3 + 1, :]
        # par=1 single: lower=tap(dy,0) odd (off 0)
        wdcp[0:64, 1, 2 * dy + 1, :] = wdc[:, dy * 3 + 0, :]
    out["wdcp"] = wdcp.astype(f16)
    bdc = np.zeros((128, 1), f32)
    for Q in range(4):
        bdc[32 * Q:32 * Q + 3, 0] = dcb
    out["bdc"] = bdc
    return out


def pos_encoding():
    c = np.arange(2, dtype=np.float32)
    yy = np.arange(64, dtype=np.float32)
    ang = yy[None, :] / (10000.0 ** (2.0 * c / 4.0)).astype(np.float32)[:, None]
    pe = np.zeros((4, 64), np.float32)
    pe[0::2] = np.sin(ang)
    pe[1::2] = np.cos(ang)
    return pe


def build_m0(x_shard, le_shard):
    """x_shard [ns,3,64,64] f32, le_shard [ns,64,64] f32 -> [ns,36,64,64] f16."""
    ns = x_shard.shape[0]
    pe = pos_encoding()
    h0 = np.zeros((ns, 4, 66, 66), np.float32)
    h0[:, :3, 1:65, 1:65] = x_shard
    h0[:, 3, 1:65, 1:65] = le_shard
    h0[:, :, 1:65, 1:65] += pe[None, :, :, None]
    m0 = np.zeros((ns, 36, 64, 64), np.float32)
    for dy in range(3):
        for dx in range(3):
            t = dy * 3 + dx
            m0[:, t * 4:t * 4 + 4] = h0[:, :, dy:dy + 64, dx:dx + 64]
    # permute columns so e1's relu write is contiguous in the h1p plane layout:
    # first 32 cols -> odd x (plane0 slots xx1..32), last 32 -> even x (plane1 xx0..31)
    m0p = np.empty_like(m0)
    m0p[:, :, :, 0:32] = m0[:, :, :, 1::2]
    m0p[:, :, :, 32:64] = m0[:, :, :, 0::2]
    return m0p.astype(np.float16)


def make_in_maps(x, labels, label_emb, static):
    le = label_emb[labels].reshape(-1, 64, 64)
    in_maps = []
    for c in range(NCORES):
        sl = slice(c * NS, (c + 1) * NS)
        m = dict(static)
        m["m0"] = build_m0(x[sl], le[sl])
        in_maps.append(m)
    return in_maps


def kernel(x, t, labels, label_emb, ew1, eb1, ew2, eb2, bw1, bb1,
           qw, qb, kw, kb, vw, vb, gamma, bw2, bb2, dtw, dtb, dcw, dcb):
    del t
    x = np.asarray(x, np.float32)
    labels = np.asarray(labels)
    label_emb = np.asarray(label_emb, np.float32)
    static = prep_static(np.asarray(ew1), np.asarray(eb1), np.asarray(ew2),
                         np.asarray(eb2), np.asarray(bw1), np.asarray(bb1),
                         np.asarray(qw), np.asarray(qb), np.asarray(kw),
                         np.asarray(kb), np.asarray(vw), np.asarray(vb),
                         np.asarray(gamma), np.asarray(bw2), np.asarray(bb2),
                         np.asarray(dtw), np.asarray(dtb), np.asarray(dcw),
                         np.asarray(dcb))
    in_maps = make_in_maps(x, labels, label_emb, static)
    if "nc" not in _cache:
        _cache["nc"] = build_nc()
    nc = _cache["nc"]
    res = run_bass_kernel_spmd(nc, in_maps, core_ids=list(range(NCORES)))
    raw = np.concatenate([res.results[c]["out"] for c in range(NCORES)], axis=0)
    out = np.empty((raw.shape[0], 3, 64, 64), np.float32)
    out[:, :, :, 0::2] = raw[:, :, 0]
    out[:, :, :, 1::2] = raw[:, :, 1]
    return out
